# revision 1
# baseline (speedup 1.0000x reference)
"""Multi-head differential attention on 8 Trainium2 NeuronCores.

Sharding: core c -> batch c//4, head-group c%4 (4 of 16 heads).
Per core: QKV projection for its heads, k-major attention (scores
transposed; softmax denominators come from a ones-row appended to V via
the AV matmul), per-(batch,head) softmax normalization and GroupNorm
statistics.  The *pre-affine* normalized tensor z is AllGathered in bf16
(pair 0's gather hides under pair 1's attention); the GroupNorm affine
(mean/rstd per head) rides the second gather as bitcast payload columns
and is folded on-device into a scaled Wo and a constant bias row.  Each
core then runs a column-parallel out-projection producing a 256-column
slice of the output, assembled on host.

Host side folds: lambda and softmax scale into Wq/bq; GroupNorm affine
into Wo/bo.  x is pre-transposed per batch and cast to bf16 so all
matmuls run at 1 cycle/row.
"""

import numpy as np
import ml_dtypes

B, S, D, H, DH = 2, 2048, 1024, 16, 64
HPC = 4            # heads per core
CW = HPC * DH      # attention columns per core (256)
EPS = 1e-5
LAMBDA_INIT = 0.8
N_CORES = 8
SCC = 16           # scalar payload columns (8 f32 as 16 bf16)

_cache = {}


def _build(with_collective=True):
    from contextlib import ExitStack
    import concourse.bass as bass
    from concourse import bacc
    import concourse.tile as tile
    import concourse.mybir as mybir

    f32 = mybir.dt.float32
    bf16 = mybir.dt.bfloat16
    AF = mybir.ActivationFunctionType
    ALU = mybir.AluOpType

    nc = bacc.Bacc("TRN2", target_bir_lowering=False, debug=False,
                   num_devices=N_CORES)

    xt_d = nc.dram_tensor("xt", [D, S], bf16, kind="ExternalInput")
    wq_d = nc.dram_tensor("wq", [D, CW], bf16, kind="ExternalInput")
    wk_d = nc.dram_tensor("wk", [D, CW], bf16, kind="ExternalInput")
    wv_d = nc.dram_tensor("wv", [D, CW], bf16, kind="ExternalInput")
    # wo: gathered-row layout [(g t o p), quarter-cols]
    wo_d = nc.dram_tensor("wo", [D, CW], bf16, kind="ExternalInput")
    bq_d = nc.dram_tensor("bq", [CW], bf16, kind="ExternalInput")
    bk_d = nc.dram_tensor("bk", [CW], bf16, kind="ExternalInput")
    bv_d = nc.dram_tensor("bv", [CW], f32, kind="ExternalInput")
    bvf_d = nc.dram_tensor("bvf", [D], f32, kind="ExternalInput")
    bo_d = nc.dram_tensor("bo", [CW], bf16, kind="ExternalInput")
    y_d = nc.dram_tensor("y", [2, 128, S], f32, kind="ExternalOutput")

    ag_in0 = nc.dram_tensor("ag_in0", [128, S], bf16)
    ag_in1 = nc.dram_tensor("ag_in1", [128, S + SCC], bf16)
    ag_out0 = nc.dram_tensor("ag_out0", [4, 128, S], bf16)
    ag_out1 = nc.dram_tensor("ag_out1", [4, 128, S + SCC], bf16)
    rs_d = nc.dram_tensor("rs_scratch", [HPC, S], f32)

    NQT = 4          # query tiles of 512
    QT = 512
    NKT = 16         # key tiles of 128
    NDC = 8          # d-chunks of 128

    with ExitStack() as ctx:
        tc = ctx.enter_context(tile.TileContext(nc))
        const = ctx.enter_context(tc.tile_pool(name="const", bufs=1))
        big = ctx.enter_context(tc.tile_pool(name="big", bufs=1))

        pxt = ctx.enter_context(tc.tile_pool(name="pxt", bufs=1))
        xt_sb = [pxt.tile([128, S], bf16, tag=f"xt{c}", name=f"xt{c}")
                 for c in range(NDC)]
        for c in range(NDC):
            nc.sync.dma_start(out=xt_sb[c], in_=xt_d[c * 128:(c + 1) * 128, :])

        # ---- constants ----
        wq_sb = const.tile([128, NDC, CW], bf16, tag="wq")
        wk_sb = const.tile([128, NDC, CW], bf16, tag="wk")
        wv_sb = const.tile([128, NDC, CW], bf16, tag="wv")
        nc.sync.dma_start(out=wq_sb, in_=wq_d[:, :].rearrange("(c p) n -> p c n", p=128))
        nc.sync.dma_start(out=wk_sb, in_=wk_d[:, :].rearrange("(c p) n -> p c n", p=128))
        nc.sync.dma_start(out=wv_sb, in_=wv_d[:, :].rearrange("(c p) n -> p c n", p=128))
        wo_sb = const.tile([128, NDC, CW], bf16, tag="wo")
        nc.sync.dma_start(out=wo_sb, in_=wo_d[:, :].rearrange("(c p) n -> p c n", p=128))

        bqr_sb = const.tile([1, CW], bf16, tag="bqr")
        bkr_sb = const.tile([1, CW], bf16, tag="bkr")
        bor_sb = const.tile([1, CW], bf16, tag="bor")
        nc.sync.dma_start(out=bqr_sb, in_=bq_d[:].rearrange("(a n) -> a n", a=1))
        nc.sync.dma_start(out=bkr_sb, in_=bk_d[:].rearrange("(a n) -> a n", a=1))
        nc.sync.dma_start(out=bor_sb, in_=bo_d[:].rearrange("(a n) -> a n", a=1))
        bv0_sb = const.tile([64, HPC], f32, tag="bv0")
        nc.sync.dma_start(out=bv0_sb, in_=bv_d[:].rearrange("(h p) -> p h", p=64))
        bv_sb = const.tile([64, HPC], f32, tag="bv")
        nc.vector.tensor_copy(bv_sb, bv0_sb)  # pre-touch: keep deps DVE-local

        onesrow_sb = const.tile([1, QT], bf16, tag="onesrow")
        nc.vector.memset(onesrow_sb, 1.0)
        ones_sb = const.tile([64, 1], f32, tag="ones")
        nc.vector.memset(ones_sb, 1.0)
        ones2_sb = const.tile([2, 64], f32, tag="ones2")
        nc.vector.memset(ones2_sb, 1.0)

        qT_sb = big.tile([128, 2, S], bf16, tag="qT")   # pair t: head 2t rows 0:64
        kT_sb = big.tile([128, 2, S], bf16, tag="kT")
        v_sb = [big.tile([128, NKT, DH + 1], bf16, tag=f"v{h}", name=f"v{h}")
                for h in range(HPC)]
        z_sb = [big.tile([DH + 1, S], f32, tag=f"z{h}", name=f"z{h}")
                for h in range(HPC)]
        zp_sb = [big.tile([128, S + SCC], bf16, tag=f"zp{t}", name=f"zp{t}")
                 for t in range(2)]

        # ---- Phase B: QKV projections (pair 0 first so attention starts early)
        with tc.tile_pool(name="pbqk", bufs=4, space="PSUM") as pbqk, \
             tc.tile_pool(name="pbv", bufs=3, space="PSUM") as pbv:

            def qk_proj(t, w_sb, br_sb, dst):
                pss = [pbqk.tile([128, QT], f32, tag="qk",
                                 name=f"qk{t}{st}{w_sb.tensor.name}")
                       for st in range(NQT)]
                for c in range(NDC):
                    for st in range(NQT):
                        nc.tensor.matmul(pss[st], w_sb[:, c, t * 128:(t + 1) * 128],
                                         xt_sb[c][:, st * QT:(st + 1) * QT],
                                         start=(c == 0), stop=False)
                for st in range(NQT):
                    nc.tensor.matmul(pss[st], br_sb[:, t * 128:(t + 1) * 128],
                                     onesrow_sb, start=False, stop=True)
                    nc.vector.tensor_copy(out=dst[:, t, st * QT:(st + 1) * QT],
                                          in_=pss[st])

            qk_proj(0, wq_sb, bqr_sb, qT_sb)
            qk_proj(0, wk_sb, bkr_sb, kT_sb)
            for st in range(NKT):
                ps = pbv.tile([128, CW], f32, tag="v", name=f"vv{st}")
                for c in range(NDC):
                    nc.tensor.matmul(ps, xt_sb[c][:, st * 128:(st + 1) * 128],
                                     wv_sb[:, c, :],
                                     start=(c == 0), stop=(c == NDC - 1))
                for h in range(HPC):
                    nc.vector.tensor_copy(out=v_sb[h][:, st, 0:DH],
                                          in_=ps[:, h * DH:(h + 1) * DH])
            for h in range(HPC):
                nc.vector.memset(v_sb[h][:, :, DH:DH + 1], 1.0)
            qk_proj(1, wq_sb, bqr_sb, qT_sb)
            qk_proj(1, wk_sb, bkr_sb, kT_sb)

        # ---- Phase C: attention per head-pair; softmax-normalize, stats,
        #      and the pair's AllGather all overlap the next pair ----
        with tc.tile_pool(name="psc", bufs=2, space="PSUM") as psc, \
             tc.tile_pool(name="pav", bufs=4, space="PSUM") as pav, \
             tc.tile_pool(name="pexp", bufs=4) as pexp, \
             tc.tile_pool(name="pd", bufs=1) as pd:
            bnst = [pd.tile([64, NQT, 6], f32, tag=f"bn{h}", name=f"bnst{h}")
                    for h in range(HPC)]
            vr_all = pd.tile([1, HPC], f32, tag="vr_all", name="vr_all")
            msc_all = pd.tile([1, 2 * HPC], f32, tag="msc", name="msc_all")
            stk_all = [pd.tile([64, 3], f32, tag=f"stk{h}", name=f"stk{h}")
                       for h in range(HPC)]
            for t in range(2):
                h0, h1 = 2 * t, 2 * t + 1
                for qt in range(NQT):
                    av0 = pav.tile([DH + 1, QT], f32, tag="av", name=f"av{t}{qt}a")
                    av1 = pav.tile([DH + 1, QT], f32, tag="av", name=f"av{t}{qt}b")
                    for kt in range(NKT):
                        sps = psc.tile([128, 2 * QT], f32, tag="s", name=f"s{t}{qt}{kt}")
                        for o in range(2):
                            nc.tensor.matmul(
                                sps[:, o * QT:(o + 1) * QT],
                                kT_sb[64 * o:64 * (o + 1), t, kt * 128:(kt + 1) * 128],
                                qT_sb[64 * o:64 * (o + 1), t, qt * QT:(qt + 1) * QT],
                                start=True, stop=True)
                        e_sb = pexp.tile([128, 2 * QT], bf16, tag="e", name=f"e{t}{qt}{kt}")
                        nc.scalar.activation(e_sb, sps, AF.Exp)
                        nc.tensor.matmul(av0, v_sb[h0][:, kt, :], e_sb[:, 0:QT],
                                         start=(kt == 0), stop=(kt == NKT - 1))
                        nc.tensor.matmul(av1, v_sb[h1][:, kt, :], e_sb[:, QT:2 * QT],
                                         start=(kt == 0), stop=(kt == NKT - 1))
                    nc.vector.tensor_copy(out=z_sb[h0][:, qt * QT:(qt + 1) * QT], in_=av0)
                    nc.vector.tensor_copy(out=z_sb[h1][:, qt * QT:(qt + 1) * QT], in_=av1)

                # softmax normalize + GN stats for this pair (DVE/DMA only:
                # no PE instructions, so pair 1's matmuls are never blocked)
                for i, h in enumerate((h0, h1)):
                    nc.sync.dma_start(out=rs_d[h:h + 1, :], in_=z_sb[h][DH:DH + 1, :])
                    rb = pd.tile([64, S], f32, tag="rb", bufs=2, name=f"rb{h}")
                    nc.gpsimd.dma_start(out=rb,
                                        in_=rs_d[h:h + 1, :].to_broadcast([64, S]))
                    nc.vector.reciprocal_approx_fast(rb, rb)
                    nc.vector.tensor_mul(z_sb[h][0:DH, :], z_sb[h][0:DH, :], rb)
                    # assemble the gather payload (bf16): even head rows 0:64
                    # via DVE cast, odd head rows 64:128 via DMA (cross-part)
                    if i == 0:
                        nc.vector.tensor_copy(out=zp_sb[t][0:64, 0:S],
                                              in_=z_sb[h][0:DH, :])
                    else:
                        nc.gpsimd.dma_start(out=zp_sb[t][64:128, 0:S],
                                            in_=z_sb[h][0:DH, :])
                    for st in range(NQT):
                        nc.vector.bn_stats(out=bnst[h][:, st, :],
                                           in_=z_sb[h][0:DH, st * QT:(st + 1) * QT])
                    mvh = pd.tile([64, 2], f32, tag="mv", bufs=2, name=f"mv{h}")
                    nc.vector.bn_aggr(out=mvh, in_=bnst[h])
                    stk = stk_all[h]
                    nc.vector.tensor_add(stk[:, 0:1], mvh[:, 0:1], bv_sb[:, h:h + 1])
                    nc.vector.tensor_copy(stk[:, 1:2], mvh[:, 1:2])
                    nc.vector.tensor_mul(stk[:, 2:3], stk[:, 0:1], stk[:, 0:1])

                if t == 0:
                    # pair 0: gather z now -- fully hidden under pair 1
                    nc.sync.dma_start(out=ag_in0[:, :], in_=zp_sb[0][:, 0:S])
                    if with_collective:
                        nc.gpsimd.collective_compute(
                            "AllGather", ALU.bypass,
                            replica_groups=[[0, 1, 2, 3], [4, 5, 6, 7]],
                            ins=[ag_in0[:].opt()],
                            outs=[ag_out0[:].opt()],
                        )
                    else:
                        for g in range(4):
                            nc.sync.dma_start(out=ag_out0[g], in_=ag_in0[:, :])

            # ---- GN scalar tail: one sqrt table switch total ----
            scg = pd.tile([1, HPC, 3], f32, tag="scg", name="scg")
            for h in range(HPC):
                stp = pav.tile([1, 3], f32, tag="av", name=f"stp{h}")
                nc.tensor.matmul(stp, ones_sb, stk_all[h], start=True, stop=True)
                nc.vector.tensor_copy(scg[:, h, :], stp)
            e3 = pd.tile([1, HPC, 3], f32, tag="e3", name="e3")
            nc.vector.tensor_scalar(out=e3, in0=scg, scalar1=1.0 / 64.0,
                                    scalar2=None, op0=ALU.mult)
            m2 = pd.tile([1, HPC], f32, tag="m2", name="m2")
            nc.vector.tensor_mul(m2, e3[:, :, 0], e3[:, :, 0])
            nc.vector.tensor_add(vr_all, e3[:, :, 1], e3[:, :, 2])
            nc.vector.tensor_tensor(out=vr_all, in0=vr_all, in1=m2, op=ALU.subtract)
            eps_t = pd.tile([1, 1], f32, tag="eps", name="eps_t")
            nc.vector.memset(eps_t, EPS)
            sd_all = pd.tile([1, HPC], f32, tag="sd", name="sd_all")
            nc.scalar.activation(sd_all, vr_all, AF.Sqrt, bias=eps_t)
            rr = pd.tile([1, HPC], f32, tag="rr", name="rr")
            nc.vector.reciprocal(rr, sd_all)
            # parity-major payload order: [M0,M2,M1,M3, r0,r2,r1,r3]
            for j, h in enumerate((0, 2, 1, 3)):
                nc.vector.tensor_copy(msc_all[:, j:j + 1], e3[:, h, 0:1])
                nc.vector.tensor_copy(msc_all[:, HPC + j:HPC + j + 1],
                                      rr[:, h:h + 1])

            # scalars ride the pair-1 gather as bitcast bf16 payload columns
            nc.vector.tensor_copy(out=zp_sb[1][0:1, S:S + SCC],
                                  in_=msc_all[0:1, :].bitcast(bf16))
            nc.sync.dma_start(out=ag_in1[:, :], in_=zp_sb[1][:, :])
            if with_collective:
                nc.gpsimd.collective_compute(
                    "AllGather", ALU.bypass,
                    replica_groups=[[0, 1, 2, 3], [4, 5, 6, 7]],
                    ins=[ag_in1[:].opt()],
                    outs=[ag_out1[:].opt()],
                )
            else:
                for g in range(4):
                    nc.sync.dma_start(out=ag_out1[g], in_=ag_in1[:, :])

        # ---- Phase E: fold GN affine into Wo, column-parallel out-proj ----
        with tc.tile_pool(name="pg", bufs=1) as pg, \
             tc.tile_pool(name="pf", bufs=4, space="PSUM") as pf, \
             tc.tile_pool(name="pystage", bufs=2) as pystage:
            # gathered scalars: [4 groups, 8 f32] as bitcast bf16 rows
            sc16 = pg.tile([1, 4, SCC], bf16, tag="sc16")
            nc.sync.dma_start(
                out=sc16,
                in_=ag_out1[:, 0:1, S:S + SCC].rearrange("g p c -> p g c"))
            # [1, 4, 8] f32 per group: (M0,M2,M1,M3, r0,r2,r1,r3)
            scf = sc16[:, :, :].bitcast(f32)
            # per out-partition half o: values for chunks c=(g,t) are the
            # parity-o pair within each group -- contiguous slices
            rstg = pg.tile([1, 2, 4, 2], f32, tag="rstg")
            mstg = pg.tile([1, 2, 4, 2], f32, tag="mstg")
            for o in range(2):
                nc.vector.tensor_copy(out=rstg[:, o, :, :],
                                      in_=scf[:, :, HPC + 2 * o:HPC + 2 * o + 2])
                nc.vector.tensor_copy(out=mstg[:, o, :, :],
                                      in_=scf[:, :, 2 * o:2 * o + 2])
            s2p = pf.tile([128, NDC], f32, tag="s2p", bufs=1, name="s2p")
            mcp = pf.tile([128, NDC], f32, tag="mcp", bufs=1, name="mcp")
            for o in range(2):
                nc.tensor.matmul(s2p[64 * o:64 * (o + 1), :], ones2_sb[0:1, :],
                                 rstg[:, o, :, :], start=True, stop=True)
                nc.tensor.matmul(mcp[64 * o:64 * (o + 1), :], ones2_sb[0:1, :],
                                 mstg[:, o, :, :], start=True, stop=True)
            s2c = pg.tile([128, NDC], f32, tag="s2c")
            nc.vector.tensor_copy(s2c, s2p)
            bvg = pg.tile([128, NDC], f32, tag="bvg")
            nc.sync.dma_start(out=bvg, in_=bvf_d[:].rearrange("(c p) -> p c", p=128))
            mcs = pg.tile([128, NDC], f32, tag="mcs")
            nc.vector.tensor_tensor(out=mcs, in0=mcp, in1=bvg, op=ALU.subtract)
            mvec = pg.tile([128, NDC], bf16, tag="mvec")
            nc.vector.tensor_mul(mvec, mcs, s2c)

            # wo_scaled[p, (c,n)] = r_head(p,c) * wo ; cst[n] = sum_p M*r*wo
            wos = pg.tile([128, NDC, CW], bf16, tag="wos")
            for c in range(NDC):
                nc.vector.tensor_scalar(out=wos[:, c, :], in0=wo_sb[:, c, :],
                                        scalar1=s2c[:, c:c + 1], scalar2=None,
                                        op0=ALU.mult)
            cstp = pf.tile([1, CW], f32, tag="cst", bufs=1, name="cstp")
            for c in range(NDC):
                nc.tensor.matmul(cstp, mvec[:, c:c + 1], wo_sb[:, c, :],
                                 start=(c == 0), stop=(c == NDC - 1))
            brow = pg.tile([1, CW], bf16, tag="brow")
            nc.vector.tensor_tensor(out=brow, in0=bor_sb, in1=cstp, op=ALU.subtract)

            nrmg_sb = pg.tile([128, NDC, S], bf16, tag="nrmg")
            for g in range(4):
                nc.sync.dma_start(out=nrmg_sb[:, 2 * g, :], in_=ag_out0[g, :, :])
                nc.sync.dma_start(out=nrmg_sb[:, 2 * g + 1, :],
                                  in_=ag_out1[g, :, 0:S])

            for nt in range(2):
                ystage = pystage.tile([128, S], f32, tag="ys", name=f"ys{nt}")
                yps = [pf.tile([128, QT], f32, tag="y", name=f"yp{nt}{st}")
                       for st in range(NQT)]
                for c in range(NDC):
                    for st in range(NQT):
                        nc.tensor.matmul(yps[st], wos[:, c, nt * 128:(nt + 1) * 128],
                                         nrmg_sb[:, c, st * QT:(st + 1) * QT],
                                         start=(c == 0), stop=False)
                for st in range(NQT):
                    nc.tensor.matmul(yps[st], brow[:, nt * 128:(nt + 1) * 128],
                                     onesrow_sb, start=False, stop=True)
                    nc.scalar.activation(ystage[:, st * QT:(st + 1) * QT], yps[st],
                                         AF.Copy)
                nc.sync.dma_start(out=y_d[nt, :, :], in_=ystage)

    nc.compile()
    return nc


def _get_nc():
    if "nc" not in _cache:
        _cache["nc"] = _build()
    return _cache["nc"]


def _host_prep(x, Wq, bq, Wk, bk, Wv, bv, Wo, bo, lq1, lk1, lq2, lk2, gn_w, gn_b):
    x = np.asarray(x, np.float32)
    lam = (np.exp((np.asarray(lq1) * np.asarray(lk1)).sum(-1))
           - np.exp((np.asarray(lq2) * np.asarray(lk2)).sum(-1)) + LAMBDA_INIT)
    qscale = (DH ** -0.5) * lam
    Wq_eff = (np.asarray(Wq).reshape(D, H, DH) * qscale[None, :, None]).reshape(D, D)
    bq_eff = (np.asarray(bq).reshape(H, DH) * qscale[:, None]).reshape(D)
    gw = np.asarray(gn_w).reshape(D)
    gb = np.asarray(gn_b).reshape(D)
    Wo_eff = np.asarray(Wo) * gw[:, None]
    bo_eff = np.asarray(bo) + gb @ np.asarray(Wo)

    # Gathered-row order (chunk (g,t), partition (o,dh) -> head 4g+2t+o) is
    # exactly the original row-major head order, so Wo_eff rows need no
    # permutation.
    xT = np.ascontiguousarray(x.transpose(0, 2, 1))  # [B, D, S]
    bf = ml_dtypes.bfloat16

    in_maps = []
    for c in range(N_CORES):
        b, hg = c // 4, c % 4
        cs = slice(CW * hg, CW * (hg + 1))
        in_maps.append({
            "xt": np.ascontiguousarray(xT[b]).astype(bf),
            "wq": np.ascontiguousarray(Wq_eff[:, cs]).astype(bf),
            "wk": np.ascontiguousarray(np.asarray(Wk)[:, cs]).astype(bf),
            "wv": np.ascontiguousarray(np.asarray(Wv)[:, cs]).astype(bf),
            "wo": np.ascontiguousarray(Wo_eff[:, cs]).astype(bf),
            "bq": np.ascontiguousarray(bq_eff[cs]).astype(bf),
            "bk": np.ascontiguousarray(np.asarray(bk)[cs]).astype(bf),
            "bv": np.ascontiguousarray(np.asarray(bv)[cs]).astype(np.float32),
            "bvf": np.ascontiguousarray(np.asarray(bv)).astype(np.float32),
            "bo": np.ascontiguousarray(bo_eff[cs]).astype(bf),
        })
    return in_maps


def _host_gather(outs):
    # core c=4b+hg produced output columns [256*hg, 256*(hg+1)) as [2,128,S]
    yT = np.empty((B, D, S), np.float32)
    for b in range(B):
        for hg in range(4):
            q = np.asarray(outs[4 * b + hg]["y"]).reshape(CW, S)
            yT[b, CW * hg:CW * (hg + 1), :] = q
    return np.ascontiguousarray(yT.transpose(0, 2, 1))


def kernel(x, Wq, bq, Wk, bk, Wv, bv, Wo, bo, lq1, lk1, lq2, lk2, gn_w, gn_b):
    from concourse.bass_utils import run_bass_kernel_spmd

    in_maps = _host_prep(x, Wq, bq, Wk, bk, Wv, bv, Wo, bo,
                         lq1, lk1, lq2, lk2, gn_w, gn_b)
    nc = _get_nc()
    res = run_bass_kernel_spmd(nc, in_maps, core_ids=list(range(N_CORES)))
    return _host_gather(res.results)



# revision 25
# speedup vs baseline: 1.0292x; 1.0292x over previous
"""Multi-head differential attention on 8 Trainium2 NeuronCores.

Sharding: core c -> batch c//4, head-group c%4 (4 of 16 heads).
Per core: QKV projection for its heads, k-major attention (scores via
row-group-packed 64-partition matmul pairs; softmax denominators from a
ones-row appended to V), per-(batch,head,qt-chunk) softmax normalization
pipelined inside the attention loop, GroupNorm statistics via bn_stats
with a DVE Newton rsqrt (no scalar-engine table switch).

Pair 0's z is GroupNorm-scaled at the source and AllGathered in one
piece (hidden under pair 1's attention).  Pair 1's z is gathered raw in
four per-qt chunks pipelined during pair 1's attention; its GN scalars
(mean/rstd) ride the last chunk as bitcast payload columns and are
folded on-device into a scaled Wo and a per-partition bias column.  The
out-projection accumulates pair-0 chunks against raw Wo while the last
gather is in flight, then finishes with pair-1 chunks.

Host side folds: lambda and softmax scale into Wq/bq; GroupNorm affine
into Wo/bo.  x is pre-transposed per batch and cast to bf16.
"""

import numpy as np
import ml_dtypes

B, S, D, H, DH = 2, 2048, 1024, 16, 64
HPC = 4            # heads per core
CW = HPC * DH      # attention columns per core (256)
EPS = 1e-5
LAMBDA_INIT = 0.8
N_CORES = 8
SCC = 8            # scalar payload columns (4 f32 as 8 bf16)
RSQRT_MAGIC = 1.32118221e+19   # f32 with bits 0x5f3759df

NQT = 4            # query tiles of 512
QT = 512
NKT = 16           # key tiles of 128
NDC = 8            # d-chunks of 128

_cache = {}


def _build(with_collective=True, debug=False):
    from contextlib import ExitStack
    import concourse.bass as bass
    from concourse import bacc
    import concourse.tile as tile
    import concourse.mybir as mybir

    f32 = mybir.dt.float32
    i32 = mybir.dt.int32
    bf16 = mybir.dt.bfloat16
    AF = mybir.ActivationFunctionType
    ALU = mybir.AluOpType

    nc = bacc.Bacc("TRN2", target_bir_lowering=False, debug=False,
                   num_devices=N_CORES)

    xt_d = nc.dram_tensor("xt", [D, S], bf16, kind="ExternalInput")
    wq_d = nc.dram_tensor("wq", [D, CW], bf16, kind="ExternalInput")
    wk_d = nc.dram_tensor("wk", [D, CW], bf16, kind="ExternalInput")
    wv_d = nc.dram_tensor("wv", [D, CW], bf16, kind="ExternalInput")
    # wo: gathered-row layout [(g t o p), quarter-cols]
    wo_d = nc.dram_tensor("wo", [D, CW], bf16, kind="ExternalInput")
    bqp_d = nc.dram_tensor("bqp", [128, 2], f32, kind="ExternalInput")
    bkp_d = nc.dram_tensor("bkp", [128, 2], f32, kind="ExternalInput")
    bvh_d = nc.dram_tensor("bvh", [64, HPC], f32, kind="ExternalInput")
    bvo_d = nc.dram_tensor("bvo", [128, HPC], f32, kind="ExternalInput")
    bo_d = nc.dram_tensor("bo", [CW], bf16, kind="ExternalInput")
    y_d = nc.dram_tensor("y", [2, 128, S], f32, kind="ExternalOutput")

    ag0_in = nc.dram_tensor("ag0_in", [128, S], bf16)
    ag0_out = nc.dram_tensor("ag0_out", [4, 128, S], bf16)
    ag1_in = [nc.dram_tensor(f"ag1_in{q}", [128, QT + (SCC if q == 3 else 0)],
                             bf16) for q in range(NQT)]
    ag1_out = [nc.dram_tensor(f"ag1_out{q}",
                              [4, 128, QT + (SCC if q == 3 else 0)], bf16)
               for q in range(NQT)]
    rb_d = nc.dram_tensor("rb_bounce", [4, QT], f32)
    if debug:
        dbgz_d = nc.dram_tensor("dbgz", [HPC, DH, S], f32, kind="ExternalOutput")
        dbgmr_d = nc.dram_tensor("dbgmr", [HPC, 2], f32, kind="ExternalOutput")
        dbgnr_d = nc.dram_tensor("dbgnr", [2, 128, 4, S], bf16, kind="ExternalOutput")
        dbgv_d = nc.dram_tensor("dbgv", [128, NKT, DH + 1], bf16, kind="ExternalOutput")
        dbgq_d = nc.dram_tensor("dbgq", [128, S], bf16, kind="ExternalOutput")
        dbgk_d = nc.dram_tensor("dbgk", [128, S], bf16, kind="ExternalOutput")
        dbge_d = nc.dram_tensor("dbge", [128, 2 * QT], bf16, kind="ExternalOutput")

    with ExitStack() as ctx:
        tc = ctx.enter_context(tile.TileContext(nc))
        const = ctx.enter_context(tc.tile_pool(name="const", bufs=1))
        big = ctx.enter_context(tc.tile_pool(name="big", bufs=1))
        psc = ctx.enter_context(tc.tile_pool(name="psc", bufs=2, space="PSUM"))
        pav = ctx.enter_context(tc.tile_pool(name="pav", bufs=2, space="PSUM"))
        ppp = ctx.enter_context(tc.tile_pool(name="ppp", bufs=2, space="PSUM"))
        pexp = ctx.enter_context(tc.tile_pool(name="pexp", bufs=4))
        pd = ctx.enter_context(tc.tile_pool(name="pd", bufs=1))
        prb = ctx.enter_context(tc.tile_pool(name="prb", bufs=2))

        # ---- input DMAs: priority order on the sync queue ----
        wq_sb = const.tile([128, NDC, CW], bf16, tag="wq")
        nc.sync.dma_start(out=wq_sb, in_=wq_d[:, :].rearrange("(c p) n -> p c n", p=128))
        pxt = ctx.enter_context(tc.tile_pool(name="pxt", bufs=1))
        xt_sb = [pxt.tile([128, S], bf16, tag=f"xt{c}", name=f"xt{c}")
                 for c in range(NDC)]
        for c in range(NDC):
            nc.sync.dma_start(out=xt_sb[c], in_=xt_d[c * 128:(c + 1) * 128, :])
        wk_sb = const.tile([128, NDC, CW], bf16, tag="wk")
        wv_sb = const.tile([128, NDC, CW], bf16, tag="wv")
        nc.sync.dma_start(out=wk_sb, in_=wk_d[:, :].rearrange("(c p) n -> p c n", p=128))
        nc.sync.dma_start(out=wv_sb, in_=wv_d[:, :].rearrange("(c p) n -> p c n", p=128))
        wo_sb = const.tile([128, NDC, CW], bf16, tag="wo")
        nc.sync.dma_start(out=wo_sb, in_=wo_d[:, :].rearrange("(c p) n -> p c n", p=128))

        # small constants on the gpsimd queue
        bqp_sb = const.tile([128, 2], f32, tag="bqp")
        bkp_sb = const.tile([128, 2], f32, tag="bkp")
        bvh_sb = const.tile([64, HPC], f32, tag="bvh")
        bvo_sb = const.tile([128, HPC], f32, tag="bvo")
        bor_sb = const.tile([1, CW], bf16, tag="bor")
        nc.gpsimd.dma_start(out=bqp_sb, in_=bqp_d[:, :])
        nc.gpsimd.dma_start(out=bkp_sb, in_=bkp_d[:, :])
        nc.gpsimd.dma_start(out=bvh_sb, in_=bvh_d[:, :])
        nc.gpsimd.dma_start(out=bvo_sb, in_=bvo_d[:, :])
        nc.gpsimd.dma_start(out=bor_sb, in_=bo_d[:].rearrange("(a n) -> a n", a=1))

        ones64 = const.tile([64, 1], f32, tag="ones64")
        nc.vector.memset(ones64, 1.0)
        ones1r = const.tile([1, 64], f32, tag="ones1r")
        nc.vector.memset(ones1r, 1.0)
        onesrow = const.tile([1, QT], bf16, tag="onesrow")
        nc.vector.memset(onesrow, 1.0)
        magic_sb = const.tile([1, 2], f32, tag="magic")
        nc.vector.memset(magic_sb, RSQRT_MAGIC)

        # pre-warm the exp activation table while DMAs stream
        warm_sb = const.tile([1, 1], f32, tag="warm")
        nc.vector.memset(warm_sb, 0.0)
        warm2 = const.tile([1, 1], f32, tag="warm2")
        nc.scalar.activation(warm2, warm_sb, AF.Exp)

        qT_sb = big.tile([128, 2, S], bf16, tag="qT")   # pair t; head-parity o rows
        kT_sb = big.tile([128, 2, S], bf16, tag="kT")
        v_sb = [big.tile([128, NKT, DH + 1], bf16, tag=f"v{h}", name=f"v{h}")
                for h in range(HPC)]
        z_sb = [big.tile([DH + 1, S], f32, tag=f"z{h}", name=f"z{h}")
                for h in range(HPC)]
        zp0_sb = big.tile([128, S], bf16, tag="zp0")
        zp1_sb = [big.tile([128, QT + SCC], bf16,
                           tag=f"zp1_{q % 2}", name=f"zp1_{q}")
                  for q in range(NQT)]
        nrmg0 = big.tile([128, 4, S], bf16, tag="nrmg0")
        nrmg1 = big.tile([128, 4, S], bf16, tag="nrmg1")

        bnst = [pd.tile([64, NQT, 6], f32, tag=f"bn{h}", name=f"bnst{h}")
                for h in range(HPC)]
        stk_all = [pd.tile([64, 3], f32, tag=f"stk{h}", name=f"stk{h}")
                   for h in range(HPC)]
        mr_sb = {}
        state = {"msc": None}

        def qk_proj_st(t, st, w_sb, bp_sb, dst):
            # one query-tile column of the q or k projection (st-serial)
            ps = ppp.tile([128, QT], f32, tag="pp",
                          name=f"qk{t}{st}{w_sb.tensor.name}")
            for c in range(NDC):
                nc.tensor.matmul(ps, w_sb[:, c, t * 128:(t + 1) * 128],
                                 xt_sb[c][:, st * QT:(st + 1) * QT],
                                 start=(c == 0), stop=(c == NDC - 1))
            nc.vector.tensor_scalar(out=dst[:, t, st * QT:(st + 1) * QT],
                                    in0=ps, scalar1=bp_sb[:, t:t + 1],
                                    scalar2=None, op0=ALU.add)

        def v_proj_st(st, h01):
            # v for heads [2*h01, 2*h01+1] at key-tile st
            cs = slice(h01 * 2 * DH, (h01 + 1) * 2 * DH)
            ps = ppp.tile([128, QT], f32, tag="pp", name=f"v{h01}{st}")
            for c in range(NDC):
                nc.tensor.matmul(ps[:, 0:2 * DH],
                                 xt_sb[c][:, st * 128:(st + 1) * 128],
                                 wv_sb[:, c, cs],
                                 start=(c == 0), stop=(c == NDC - 1))
            for hh in range(2):
                h = 2 * h01 + hh
                nc.vector.tensor_copy(out=v_sb[h][:, st, 0:DH],
                                      in_=ps[:, hh * DH:(hh + 1) * DH])

        def pair_stats(t):
            # bn_aggr + cross-partition combine + Newton rsqrt -> (M', r)
            h0, h1 = 2 * t, 2 * t + 1
            scg = pd.tile([1, 2, 3], f32, tag=f"scg{t}", name=f"scg{t}")
            for i, h in enumerate((h0, h1)):
                mvh = pd.tile([64, 2], f32, tag="mv", bufs=2, name=f"mv{h}")
                nc.vector.bn_aggr(out=mvh, in_=bnst[h])
                stk = stk_all[h]
                nc.vector.tensor_add(stk[:, 0:1], mvh[:, 0:1], bvh_sb[:, h:h + 1])
                nc.vector.tensor_copy(stk[:, 1:2], mvh[:, 1:2])
                nc.vector.tensor_mul(stk[:, 2:3], stk[:, 0:1], stk[:, 0:1])
                stp = ppp.tile([1, 3], f32, tag="pp", name=f"stp{h}")
                nc.tensor.matmul(stp, ones64, stk, start=True, stop=True)
                nc.vector.tensor_scalar(out=scg[:, i, :], in0=stp,
                                        scalar1=1.0 / 64.0, scalar2=None,
                                        op0=ALU.mult)
            # var_tot = E[var] + E[(m+bv)^2] - M'^2 ; r = rsqrt(var_tot + eps)
            m2 = pd.tile([1, 2], f32, tag=f"m2{t}", name=f"m2{t}")
            nc.vector.tensor_mul(m2, scg[:, :, 0], scg[:, :, 0])
            vr = pd.tile([1, 2], f32, tag=f"vr{t}", name=f"vr{t}")
            nc.vector.tensor_add(vr, scg[:, :, 1], scg[:, :, 2])
            nc.vector.tensor_tensor(out=vr, in0=vr, in1=m2, op=ALU.subtract)
            nc.vector.tensor_scalar(out=vr, in0=vr, scalar1=EPS, scalar2=None,
                                    op0=ALU.add)
            yr = pd.tile([1, 2], f32, tag=f"yr{t}", name=f"yr{t}")
            ish = pd.tile([1, 2], i32, tag=f"ish{t}", name=f"ish{t}")
            nc.vector.tensor_scalar(out=ish, in0=vr[:, :].bitcast(i32),
                                    scalar1=1, scalar2=None,
                                    op0=ALU.logical_shift_right)
            nc.vector.tensor_tensor(
                out=yr[:, :].bitcast(i32), in0=magic_sb[:, :].bitcast(i32),
                in1=ish, op=ALU.subtract)
            tt = pd.tile([1, 2], f32, tag=f"tt{t}", name=f"tt{t}")
            for _ in range(3):
                nc.vector.tensor_mul(tt, yr, yr)
                nc.vector.tensor_mul(tt, tt, vr)
                nc.vector.tensor_scalar(out=tt, in0=tt, scalar1=-0.5,
                                        scalar2=1.5, op0=ALU.mult, op1=ALU.add)
                nc.vector.tensor_mul(yr, yr, tt)
            for i, h in enumerate((h0, h1)):
                mr = pd.tile([1, 2], f32, tag="mr", bufs=4, name=f"mr{h}")
                nc.vector.tensor_copy(mr[:, 0:1], scg[:, i, 0:1])
                nc.vector.tensor_copy(mr[:, 1:2], yr[:, i:i + 1])
                mr_sb[h] = mr
            if t == 1:
                msc = pd.tile([1, 4], f32, tag="msc", name="msc")
                for i, h in enumerate((h0, h1)):
                    nc.vector.tensor_copy(msc[:, 2 * i:2 * i + 1],
                                          mr_sb[h][:, 0:1])
                    nc.vector.tensor_copy(msc[:, 2 * i + 1:2 * i + 2],
                                          mr_sb[h][:, 1:2])
                state["msc"] = msc

        qk1_work = []
        for st in range(NQT):
            qk1_work.append(("q", st))
            qk1_work.append(("k", st))

        def attn_qt(t, qt, sprinkle):
            h0, h1 = 2 * t, 2 * t + 1
            av0 = pav.tile([DH + 1, QT], f32, tag="av", name=f"av{t}{qt}a")
            av1 = pav.tile([DH + 1, QT], f32, tag="av", name=f"av{t}{qt}b")
            for kt in range(NKT):
                sps = psc.tile([128, 2 * QT], f32, tag="s", name=f"s{t}{qt}{kt}")
                for o in range(2):
                    nc.tensor.matmul(
                        sps[:, o * QT:(o + 1) * QT],
                        kT_sb[64 * o:64 * (o + 1), t, kt * 128:(kt + 1) * 128],
                        qT_sb[64 * o:64 * (o + 1), t, qt * QT:(qt + 1) * QT],
                        start=True, stop=True)
                e_sb = pexp.tile([128, 2 * QT], bf16, tag="e", name=f"e{t}{qt}{kt}")
                nc.scalar.activation(e_sb, sps, AF.Exp)
                if debug and t == 0 and qt == 0 and kt == 0:
                    nc.sync.dma_start(out=dbge_d[:, :], in_=e_sb)
                nc.tensor.matmul(av0, v_sb[h0][:, kt, :], e_sb[:, 0:QT],
                                 start=(kt == 0), stop=(kt == NKT - 1))
                nc.tensor.matmul(av1, v_sb[h1][:, kt, :], e_sb[:, QT:2 * QT],
                                 start=(kt == 0), stop=(kt == NKT - 1))
                if sprinkle == "v1" and kt % 2 == 0:
                    v_proj_st(qt * 8 + kt // 2, 1)
                elif sprinkle == "qk1" and kt % 4 == 0:
                    kind, pst = qk1_work.pop(0)
                    if kind == "q":
                        qk_proj_st(1, pst, wq_sb, bqp_sb, qT_sb)
                    else:
                        qk_proj_st(1, pst, wk_sb, bkp_sb, kT_sb)
            # per-qt softmax normalization (overlaps next qt's attention)
            for i, (h, av) in enumerate(((h0, av0), (h1, av1))):
                row = 2 * (qt % 2) + i
                zsl = z_sb[h][:, qt * QT:(qt + 1) * QT]
                nc.vector.tensor_copy(out=zsl, in_=av)
                nc.sync.dma_start(out=rb_d[row:row + 1, :],
                                  in_=z_sb[h][DH:DH + 1, qt * QT:(qt + 1) * QT])
                rb = prb.tile([64, QT], f32, tag="rb", name=f"rb{t}{qt}{i}")
                nc.gpsimd.dma_start(out=rb,
                                    in_=rb_d[row:row + 1, :].to_broadcast([64, QT]))
                nc.vector.reciprocal_approx_fast(rb, rb)
                nc.vector.tensor_mul(zsl[0:DH, :], zsl[0:DH, :], rb)
                nc.vector.bn_stats(out=bnst[h][:, qt, :], in_=zsl[0:DH, :])
            if t == 1:
                # assemble + gather this qt chunk of raw pair-1 z
                zp = zp1_sb[qt]
                nc.vector.tensor_copy(out=zp[0:64, 0:QT],
                                      in_=z_sb[h0][0:DH, qt * QT:(qt + 1) * QT])
                nc.gpsimd.dma_start(out=zp[64:128, 0:QT],
                                    in_=z_sb[h1][0:DH, qt * QT:(qt + 1) * QT])
                if qt == 3:
                    pair_stats(1)   # payload must be ready before the DMA
                    nc.vector.tensor_copy(out=zp[0:1, QT:QT + SCC],
                                          in_=state["msc"][0:1, :].bitcast(bf16))
                    nc.sync.dma_start(out=ag1_in[qt][:, :], in_=zp[:, :])
                else:
                    nc.sync.dma_start(out=ag1_in[qt][:, :], in_=zp[:, 0:QT])
                if with_collective:
                    nc.gpsimd.collective_compute(
                        "AllGather", ALU.bypass,
                        replica_groups=[[0, 1, 2, 3], [4, 5, 6, 7]],
                        ins=[ag1_in[qt][:].opt()],
                        outs=[ag1_out[qt][:].opt()],
                    )
                else:
                    for g in range(4):
                        nc.sync.dma_start(out=ag1_out[qt][g], in_=ag1_in[qt][:, :])

        # ---- lead: q/k for pair 0, v for heads 0-1 ----
        for st in range(NQT):
            qk_proj_st(0, st, wq_sb, bqp_sb, qT_sb)
            qk_proj_st(0, st, wk_sb, bkp_sb, kT_sb)
        for st in range(NKT):
            v_proj_st(st, 0)
        for h in range(2):
            nc.vector.memset(v_sb[h][:, :, DH:DH + 1], 1.0)

        # ---- pair 0 attention (v1 sprinkled in qt0-1, qk1 in qt2-3) ----
        attn_qt(0, 0, "v1")
        attn_qt(0, 1, "v1")
        for h in range(2, 4):
            nc.vector.memset(v_sb[h][:, :, DH:DH + 1], 1.0)
        attn_qt(0, 2, "qk1")
        attn_qt(0, 3, "qk1")

        # ---- pair-0 stats; source-side GN scale; gather (hidden) ----
        pair_stats(0)
        if debug:
            nc.sync.dma_start(out=dbgz_d[0], in_=z_sb[0][0:DH, :])
            nc.sync.dma_start(out=dbgz_d[1], in_=z_sb[1][0:DH, :])
            nc.sync.dma_start(out=dbgv_d[:, :, :], in_=v_sb[0])
            nc.sync.dma_start(out=dbgq_d[:, :], in_=qT_sb[:, 0, :])
            nc.sync.dma_start(out=dbgk_d[:, :], in_=kT_sb[:, 0, :])
        for i, h in enumerate((0, 1)):
            mrp = ppp.tile([64, 2], f32, tag="pp", name=f"mrp{h}")
            nc.tensor.matmul(mrp, ones1r, mr_sb[h], start=True, stop=True)
            s1 = pd.tile([64, 1], f32, tag="s1", bufs=2, name=f"s1{h}")
            nc.vector.tensor_tensor(out=s1, in0=bvh_sb[:, h:h + 1],
                                    in1=mrp[:, 0:1], op=ALU.subtract)
            s2 = pd.tile([64, 1], f32, tag="s2", bufs=2, name=f"s2{h}")
            nc.vector.tensor_copy(s2, mrp[:, 1:2])
            if i == 0:
                nc.vector.tensor_scalar(out=zp0_sb[0:64, :], in0=z_sb[h][0:DH, :],
                                        scalar1=s1, scalar2=s2,
                                        op0=ALU.add, op1=ALU.mult)
            else:
                nc.vector.tensor_scalar(out=z_sb[h][0:DH, :], in0=z_sb[h][0:DH, :],
                                        scalar1=s1, scalar2=s2,
                                        op0=ALU.add, op1=ALU.mult)
                nc.gpsimd.dma_start(out=zp0_sb[64:128, :], in_=z_sb[h][0:DH, :])
        nc.sync.dma_start(out=ag0_in[:, :], in_=zp0_sb[:, :])
        if with_collective:
            nc.gpsimd.collective_compute(
                "AllGather", ALU.bypass,
                replica_groups=[[0, 1, 2, 3], [4, 5, 6, 7]],
                ins=[ag0_in[:].opt()],
                outs=[ag0_out[:].opt()],
            )
        else:
            for g in range(4):
                nc.sync.dma_start(out=ag0_out[g], in_=ag0_in[:, :])

        # ---- pair 1 attention with per-qt chunked gathers ----
        attn_qt(1, 0, None)
        attn_qt(1, 1, None)
        # stage gathered pair-0 z mid-pair-1 (gpsimd queue has slack)
        nc.gpsimd.dma_start(out=nrmg0,
                            in_=ag0_out[:, :, :].rearrange("g p s -> p g s"))
        attn_qt(1, 2, None)
        attn_qt(1, 3, None)

        if debug:
            nc.sync.dma_start(out=dbgz_d[2], in_=z_sb[2][0:DH, :])
            nc.sync.dma_start(out=dbgz_d[3], in_=z_sb[3][0:DH, :])
            for h in range(HPC):
                nc.sync.dma_start(out=dbgmr_d[h:h + 1, :], in_=mr_sb[h])

        # staging of gathered pair-1 chunks (scalar queue: free post-attention)
        for qt in range(NQT):
            nc.scalar.dma_start(out=nrmg1[:, :, qt * QT:(qt + 1) * QT],
                                in_=ag1_out[qt][:, :, 0:QT].rearrange("g p q -> p g q"))
        sc16 = pd.tile([1, 4, SCC], bf16, tag="sc16", name="sc16")
        nc.scalar.dma_start(
            out=sc16,
            in_=ag1_out[3][:, 0:1, QT:QT + SCC].rearrange("g p c -> p g c"))

        if debug:
            nc.sync.dma_start(out=dbgnr_d[0], in_=nrmg0)
            nc.sync.dma_start(out=dbgnr_d[1], in_=nrmg1)

        # ---- tail: out-projection ----
        with tc.tile_pool(name="pg", bufs=1) as pg, \
             tc.tile_pool(name="pystage", bufs=1) as pystage:
            ystage = [pystage.tile([128, S], f32, tag=f"ys{nt}", name=f"ys{nt}")
                      for nt in range(2)]
            # nt0 pair-0 accumulation runs while the last gather is in flight
            yp0 = [psc.tile([128, 2 * QT], f32, tag="s", name=f"yp0_{j}")
                   for j in range(2)]
            for g in range(4):
                for st in range(NQT):
                    nc.tensor.matmul(
                        yp0[st // 2][:, (st % 2) * QT:(st % 2 + 1) * QT],
                        wo_sb[:, 2 * g, 0:128],
                        nrmg0[:, g, st * QT:(st + 1) * QT],
                        start=(g == 0), stop=False)
            # nt1 st0-1 pair-0 accumulation (pp-tag psum)
            yp1a = [ppp.tile([128, QT], f32, tag="pp", name=f"yp1a_{st}")
                    for st in range(2)]
            for g in range(4):
                for st in range(2):
                    nc.tensor.matmul(
                        yp1a[st],
                        wo_sb[:, 2 * g, 128:256],
                        nrmg0[:, g, st * QT:(st + 1) * QT],
                        start=(g == 0), stop=False)

            # receiver-side scalar maps (waits on the last gather)
            scf = sc16[:, :, :].bitcast(f32)       # [1, 4, 4]: M2 r2 M3 r3
            mapp = pav.tile([DH + 1, QT], f32, tag="av", name="mapp")
            for o in range(2):
                nc.tensor.matmul(mapp[0:64, 8 * o:8 * o + 4], ones1r,
                                 scf[:, :, 2 * o], start=True, stop=True)
                nc.tensor.matmul(mapp[0:64, 8 * o + 4:8 * o + 8], ones1r,
                                 scf[:, :, 2 * o + 1], start=True, stop=True)
            mapm = pg.tile([128, 4], f32, tag="mapm")
            mapr = pg.tile([128, 4], f32, tag="mapr")
            for o in range(2):
                nc.vector.tensor_copy(mapm[64 * o:64 * (o + 1), :],
                                      mapp[0:64, 8 * o:8 * o + 4])
                nc.vector.tensor_copy(mapr[64 * o:64 * (o + 1), :],
                                      mapp[0:64, 8 * o + 4:8 * o + 8])
            mvec1 = pg.tile([128, 4], bf16, tag="mvec1")
            mtmp = pg.tile([128, 4], f32, tag="mtmp")
            nc.vector.tensor_tensor(out=mtmp, in0=bvo_sb, in1=mapm,
                                    op=ALU.subtract)
            nc.vector.tensor_mul(mvec1, mtmp, mapr)
            wos1 = pg.tile([128, 4, CW], bf16, tag="wos1")
            for g in range(4):
                nc.vector.tensor_scalar(out=wos1[:, g, :],
                                        in0=wo_sb[:, 2 * g + 1, :],
                                        scalar1=mapr[:, g:g + 1], scalar2=None,
                                        op0=ALU.mult)
            # bias row: bo + sum_d (bv-M)*r*wo over pair-1 rows
            cstp = pav.tile([1, CW], f32, tag="av", name="cstp")
            for g in range(4):
                nc.tensor.matmul(cstp, mvec1[:, g:g + 1], wo_sb[:, 2 * g + 1, :],
                                 start=(g == 0), stop=(g == 3))
            brow = pg.tile([1, CW], bf16, tag="brow")
            nc.vector.tensor_tensor(out=brow, in0=bor_sb, in1=cstp, op=ALU.add)

            # nt0 pair-1 accumulation + bias + drain
            for g in range(4):
                for st in range(NQT):
                    nc.tensor.matmul(
                        yp0[st // 2][:, (st % 2) * QT:(st % 2 + 1) * QT],
                        wos1[:, g, 0:128],
                        nrmg1[:, g, st * QT:(st + 1) * QT],
                        start=False, stop=False)
            for st in range(NQT):
                src = yp0[st // 2][:, (st % 2) * QT:(st % 2 + 1) * QT]
                nc.tensor.matmul(src, brow[:, 0:128], onesrow,
                                 start=False, stop=True)
                dst = ystage[0][:, st * QT:(st + 1) * QT]
                if st % 2 == 0:
                    nc.scalar.activation(dst, src, AF.Copy)
                else:
                    nc.vector.tensor_copy(out=dst, in_=src)
            nc.sync.dma_start(out=y_d[0, :, :], in_=ystage[0])

            # nt1: st0-1 pair-1 + drain; st2-3 full accumulation + drain
            for g in range(4):
                for st in range(2):
                    nc.tensor.matmul(yp1a[st], wos1[:, g, 128:256],
                                     nrmg1[:, g, st * QT:(st + 1) * QT],
                                     start=False, stop=False)
            for st in range(2):
                nc.tensor.matmul(yp1a[st], brow[:, 128:256], onesrow,
                                 start=False, stop=True)
            yp1b = psc.tile([128, 2 * QT], f32, tag="s", name="yp1b")
            for g in range(4):
                for st in range(2, NQT):
                    nc.tensor.matmul(
                        yp1b[:, (st - 2) * QT:(st - 1) * QT],
                        wo_sb[:, 2 * g, 128:256],
                        nrmg0[:, g, st * QT:(st + 1) * QT],
                        start=(g == 0), stop=False)
            for g in range(4):
                for st in range(2, NQT):
                    nc.tensor.matmul(
                        yp1b[:, (st - 2) * QT:(st - 1) * QT],
                        wos1[:, g, 128:256],
                        nrmg1[:, g, st * QT:(st + 1) * QT],
                        start=False, stop=False)
            for st in range(2, NQT):
                nc.tensor.matmul(yp1b[:, (st - 2) * QT:(st - 1) * QT],
                                 brow[:, 128:256], onesrow,
                                 start=False, stop=True)
            for st in range(NQT):
                src = (yp1a[st] if st < 2
                       else yp1b[:, (st - 2) * QT:(st - 1) * QT])
                dst = ystage[1][:, st * QT:(st + 1) * QT]
                if st % 2 == 0:
                    nc.scalar.activation(dst, src, AF.Copy)
                else:
                    nc.vector.tensor_copy(out=dst, in_=src)
                nc.sync.dma_start(out=y_d[1, :, st * QT:(st + 1) * QT],
                                  in_=ystage[1][:, st * QT:(st + 1) * QT])

    nc.compile()
    return nc


def _get_nc():
    if "nc" not in _cache:
        _cache["nc"] = _build()
    return _cache["nc"]


def _host_prep(x, Wq, bq, Wk, bk, Wv, bv, Wo, bo, lq1, lk1, lq2, lk2, gn_w, gn_b):
    x = np.asarray(x, np.float32)
    lam = (np.exp((np.asarray(lq1) * np.asarray(lk1)).sum(-1))
           - np.exp((np.asarray(lq2) * np.asarray(lk2)).sum(-1)) + LAMBDA_INIT)
    qscale = (DH ** -0.5) * lam
    Wq_eff = (np.asarray(Wq).reshape(D, H, DH) * qscale[None, :, None]).reshape(D, D)
    bq_eff = (np.asarray(bq).reshape(H, DH) * qscale[:, None]).reshape(D)
    gw = np.asarray(gn_w).reshape(D)
    gb = np.asarray(gn_b).reshape(D)
    Wo_eff = np.asarray(Wo) * gw[:, None]
    bo_eff = np.asarray(bo) + gb @ np.asarray(Wo)
    bk_full = np.asarray(bk)
    bv_full = np.asarray(bv, np.float32)

    # Gathered-row order (chunk (g,t), partition (o,dh) -> head 4g+2t+o) is
    # exactly the original row-major head order, so Wo_eff rows need no
    # permutation.
    xT = np.ascontiguousarray(x.transpose(0, 2, 1))  # [B, D, S]
    bf = ml_dtypes.bfloat16

    def pair_partition_layout(vec256):
        # [256] (head-major: (2t+o)*64+dh) -> [128, 2] with row o*64+dh, col t
        return np.ascontiguousarray(
            vec256.reshape(2, 2, DH).transpose(1, 2, 0).reshape(128, 2)
        ).astype(np.float32)

    # receiver-side bv map for pair-1 chunks: bvo[o*64+dh, g] = bv[(4g+2+o)*64+dh]
    bvo = np.ascontiguousarray(
        bv_full.reshape(4, 2, 2, DH)[:, 1].transpose(1, 2, 0).reshape(128, 4)
    ).astype(np.float32)

    in_maps = []
    for c in range(N_CORES):
        b, hg = c // 4, c % 4
        cs = slice(CW * hg, CW * (hg + 1))
        in_maps.append({
            "xt": np.ascontiguousarray(xT[b]).astype(bf),
            "wq": np.ascontiguousarray(Wq_eff[:, cs]).astype(bf),
            "wk": np.ascontiguousarray(np.asarray(Wk)[:, cs]).astype(bf),
            "wv": np.ascontiguousarray(np.asarray(Wv)[:, cs]).astype(bf),
            "wo": np.ascontiguousarray(Wo_eff[:, cs]).astype(bf),
            "bqp": pair_partition_layout(bq_eff[cs]),
            "bkp": pair_partition_layout(bk_full[cs]),
            "bvh": np.ascontiguousarray(
                bv_full[cs].reshape(HPC, DH).T).astype(np.float32),
            "bvo": bvo,
            "bo": np.ascontiguousarray(bo_eff[cs]).astype(bf),
        })
    return in_maps


def _host_gather(outs):
    # core c=4b+hg produced output columns [256*hg, 256*(hg+1)) as [2,128,S]
    yT = np.empty((B, D, S), np.float32)
    for b in range(B):
        for hg in range(4):
            q = np.asarray(outs[4 * b + hg]["y"]).reshape(CW, S)
            yT[b, CW * hg:CW * (hg + 1), :] = q
    return np.ascontiguousarray(yT.transpose(0, 2, 1))


def kernel(x, Wq, bq, Wk, bk, Wv, bv, Wo, bo, lq1, lk1, lq2, lk2, gn_w, gn_b):
    from concourse.bass_utils import run_bass_kernel_spmd

    in_maps = _host_prep(x, Wq, bq, Wk, bk, Wv, bv, Wo, bo,
                         lq1, lk1, lq2, lk2, gn_w, gn_b)
    nc = _get_nc()
    res = run_bass_kernel_spmd(nc, in_maps, core_ids=list(range(N_CORES)))
    return _host_gather(res.results)


# revision 28
# speedup vs baseline: 1.0738x; 1.0434x over previous
"""Multi-head differential attention on 8 Trainium2 NeuronCores.

Sharding: core c -> batch c//4, head-group c%4 (4 of 16 heads).
Per core: QKV projection for its heads (pair-1 q/k and v sprinkled into
pair-0's exp-bound attention loop), k-major attention (scores via
row-group-packed 64-partition matmul pairs; softmax denominators from a
ones-row appended to V), per-(batch,head,qt-chunk) softmax normalization
pipelined inside the attention loop, GroupNorm statistics via bn_stats
with a DVE Newton rsqrt (no scalar-engine table switch).

Raw (pre-GroupNorm) z is AllGathered in eight per-qt [128,512] chunks
pipelined across both pairs' attention so the slow collective fabric
streams continuously.  Each pair's GN scalars (mean/rstd per head)
travel as bitcast payload: pair 0's ride its last chunk, pair 1's go in
a tiny dedicated gather fired before the last chunk so the Wo-scaling
fold overlaps the final transfer.  The receiver folds (bv-M)*r into a
scaled Wo and a bias row; the out-projection accumulates pair-0 chunks
while the last gather is in flight and finishes per seq-tile.

Host side folds: lambda and softmax scale into Wq/bq; GroupNorm affine
into Wo/bo.  x is pre-transposed per batch and cast to bf16.
"""

import numpy as np
import ml_dtypes

B, S, D, H, DH = 2, 2048, 1024, 16, 64
HPC = 4            # heads per core
CW = HPC * DH      # attention columns per core (256)
EPS = 1e-5
LAMBDA_INIT = 0.8
N_CORES = 8
SCC = 8            # scalar payload columns (4 f32 as 8 bf16)
RSQRT_MAGIC = 1.32118221e+19   # f32 with bits 0x5f3759df

NQT = 4            # query tiles of 512
QT = 512
NKT = 16           # key tiles of 128
NDC = 8            # d-chunks of 128
RG = [[0, 1, 2, 3], [4, 5, 6, 7]]

_cache = {}


def _build(with_collective=True, debug=False):
    from contextlib import ExitStack
    import concourse.bass as bass
    from concourse import bacc
    import concourse.tile as tile
    import concourse.mybir as mybir

    f32 = mybir.dt.float32
    i32 = mybir.dt.int32
    bf16 = mybir.dt.bfloat16
    AF = mybir.ActivationFunctionType
    ALU = mybir.AluOpType

    nc = bacc.Bacc("TRN2", target_bir_lowering=False, debug=False,
                   num_devices=N_CORES)

    xt_d = nc.dram_tensor("xt", [D, S], bf16, kind="ExternalInput")
    wq_d = nc.dram_tensor("wq", [D, CW], bf16, kind="ExternalInput")
    wk_d = nc.dram_tensor("wk", [D, CW], bf16, kind="ExternalInput")
    wv_d = nc.dram_tensor("wv", [D, CW], bf16, kind="ExternalInput")
    # wo: gathered-row layout [(g t o p), quarter-cols]
    wo_d = nc.dram_tensor("wo", [D, CW], bf16, kind="ExternalInput")
    bqp_d = nc.dram_tensor("bqp", [128, 2], f32, kind="ExternalInput")
    bkp_d = nc.dram_tensor("bkp", [128, 2], f32, kind="ExternalInput")
    bvh_d = nc.dram_tensor("bvh", [64, HPC], f32, kind="ExternalInput")
    bvo_d = nc.dram_tensor("bvo", [128, 2 * HPC], f32, kind="ExternalInput")
    bo_d = nc.dram_tensor("bo", [CW], bf16, kind="ExternalInput")
    y_d = nc.dram_tensor("y", [2, 128, S], f32, kind="ExternalOutput")

    # per-(pair,qt) chunk gathers; pair-0 qt3 carries the payload columns
    def chunk_w(t, q):
        return QT + (SCC if (t == 0 and q == 3) else 0)
    agc_in = [[nc.dram_tensor(f"agc_in{t}{q}", [128, chunk_w(t, q)], bf16)
               for q in range(NQT)] for t in range(2)]
    agc_out = [[nc.dram_tensor(f"agc_out{t}{q}", [4, 128, chunk_w(t, q)], bf16)
                for q in range(NQT)] for t in range(2)]
    sc1_in = nc.dram_tensor("sc1_in", [1, SCC], bf16)
    sc1_out = nc.dram_tensor("sc1_out", [4, 1, SCC], bf16)
    rb_d = nc.dram_tensor("rb_bounce", [4, QT], f32)
    if debug:
        dbgz_d = nc.dram_tensor("dbgz", [HPC, DH, S], f32, kind="ExternalOutput")
        dbgmr_d = nc.dram_tensor("dbgmr", [HPC, 2], f32, kind="ExternalOutput")
        dbgnr_d = nc.dram_tensor("dbgnr", [2, 128, 4, S], bf16, kind="ExternalOutput")

    with ExitStack() as ctx:
        tc = ctx.enter_context(tile.TileContext(nc))
        const = ctx.enter_context(tc.tile_pool(name="const", bufs=1))
        big = ctx.enter_context(tc.tile_pool(name="big", bufs=1))
        psc = ctx.enter_context(tc.tile_pool(name="psc", bufs=2, space="PSUM"))
        pav = ctx.enter_context(tc.tile_pool(name="pav", bufs=2, space="PSUM"))
        ppp = ctx.enter_context(tc.tile_pool(name="ppp", bufs=2, space="PSUM"))
        pexp = ctx.enter_context(tc.tile_pool(name="pexp", bufs=4))
        pd = ctx.enter_context(tc.tile_pool(name="pd", bufs=1))
        prb = ctx.enter_context(tc.tile_pool(name="prb", bufs=2))

        # ---- input DMAs: priority order on the sync queue ----
        wq_sb = const.tile([128, NDC, CW], bf16, tag="wq")
        nc.sync.dma_start(out=wq_sb, in_=wq_d[:, :].rearrange("(c p) n -> p c n", p=128))
        pxt = ctx.enter_context(tc.tile_pool(name="pxt", bufs=1))
        xt_sb = [pxt.tile([128, S], bf16, tag=f"xt{c}", name=f"xt{c}")
                 for c in range(NDC)]
        for c in range(NDC):
            nc.sync.dma_start(out=xt_sb[c], in_=xt_d[c * 128:(c + 1) * 128, :])
        wk_sb = const.tile([128, NDC, CW], bf16, tag="wk")
        wv_sb = const.tile([128, NDC, CW], bf16, tag="wv")
        nc.sync.dma_start(out=wk_sb, in_=wk_d[:, :].rearrange("(c p) n -> p c n", p=128))
        nc.sync.dma_start(out=wv_sb, in_=wv_d[:, :].rearrange("(c p) n -> p c n", p=128))
        wo_sb = const.tile([128, NDC, CW], bf16, tag="wo")
        nc.sync.dma_start(out=wo_sb, in_=wo_d[:, :].rearrange("(c p) n -> p c n", p=128))

        # small constants on the gpsimd queue
        bqp_sb = const.tile([128, 2], f32, tag="bqp")
        bkp_sb = const.tile([128, 2], f32, tag="bkp")
        bvh_sb = const.tile([64, HPC], f32, tag="bvh")
        bvo_sb = const.tile([128, 2 * HPC], f32, tag="bvo")
        bor_sb = const.tile([1, CW], bf16, tag="bor")
        nc.gpsimd.dma_start(out=bqp_sb, in_=bqp_d[:, :])
        nc.gpsimd.dma_start(out=bkp_sb, in_=bkp_d[:, :])
        nc.gpsimd.dma_start(out=bvh_sb, in_=bvh_d[:, :])
        nc.gpsimd.dma_start(out=bvo_sb, in_=bvo_d[:, :])
        nc.gpsimd.dma_start(out=bor_sb, in_=bo_d[:].rearrange("(a n) -> a n", a=1))

        ones64 = const.tile([64, 1], f32, tag="ones64")
        nc.vector.memset(ones64, 1.0)
        ones1r = const.tile([1, 64], f32, tag="ones1r")
        nc.vector.memset(ones1r, 1.0)
        onesrow = const.tile([1, QT], bf16, tag="onesrow")
        nc.vector.memset(onesrow, 1.0)
        magic_sb = const.tile([1, 2], f32, tag="magic")
        nc.vector.memset(magic_sb, RSQRT_MAGIC)

        # pre-warm the exp activation table while DMAs stream
        warm_sb = const.tile([1, 1], f32, tag="warm")
        nc.vector.memset(warm_sb, 0.0)
        warm2 = const.tile([1, 1], f32, tag="warm2")
        nc.scalar.activation(warm2, warm_sb, AF.Exp)

        qT_sb = big.tile([128, 2, S], bf16, tag="qT")   # pair t; head-parity o rows
        kT_sb = big.tile([128, 2, S], bf16, tag="kT")
        v_sb = [big.tile([128, NKT, DH + 1], bf16, tag=f"v{h}", name=f"v{h}")
                for h in range(HPC)]
        z_sb = [big.tile([DH, S], bf16, tag=f"z{h}", name=f"z{h}")
                for h in range(HPC)]
        nrmg = big.tile([128, NDC, S], bf16, tag="nrmg")   # chunk c=2g+t

        bnst = [pd.tile([64, NQT, 6], f32, tag=f"bn{h}", name=f"bnst{h}")
                for h in range(HPC)]
        stk_all = [pd.tile([64, 3], f32, tag=f"stk{h}", name=f"stk{h}")
                   for h in range(HPC)]
        mr_sb = {}
        state = {}

        def qk_proj_pair(t, st0, w_sb, bp_sb, dst):
            # two query-tile columns, c-outer so work starts on xt chunk 0
            ps = [ppp.tile([128, QT], f32, tag="pp",
                           name=f"qk{t}{st0}{j}{w_sb.tensor.name}")
                  for j in range(2)]
            for c in range(NDC):
                for j in range(2):
                    nc.tensor.matmul(ps[j], w_sb[:, c, t * 128:(t + 1) * 128],
                                     xt_sb[c][:, (st0 + j) * QT:(st0 + j + 1) * QT],
                                     start=(c == 0), stop=(c == NDC - 1))
            for j in range(2):
                nc.vector.tensor_scalar(
                    out=dst[:, t, (st0 + j) * QT:(st0 + j + 1) * QT],
                    in0=ps[j], scalar1=bp_sb[:, t:t + 1],
                    scalar2=None, op0=ALU.add)

        def qk_proj_st(t, st, w_sb, bp_sb, dst):
            ps = ppp.tile([128, QT], f32, tag="pp",
                          name=f"qk{t}{st}{w_sb.tensor.name}")
            for c in range(NDC):
                nc.tensor.matmul(ps, w_sb[:, c, t * 128:(t + 1) * 128],
                                 xt_sb[c][:, st * QT:(st + 1) * QT],
                                 start=(c == 0), stop=(c == NDC - 1))
            nc.vector.tensor_scalar(out=dst[:, t, st * QT:(st + 1) * QT],
                                    in0=ps, scalar1=bp_sb[:, t:t + 1],
                                    scalar2=None, op0=ALU.add)

        def v_proj_st(st, h01):
            # v for heads [2*h01, 2*h01+1] at key-tile st
            cs = slice(h01 * 2 * DH, (h01 + 1) * 2 * DH)
            ps = ppp.tile([128, QT], f32, tag="pp", name=f"v{h01}{st}")
            for c in range(NDC):
                nc.tensor.matmul(ps[:, 0:2 * DH],
                                 xt_sb[c][:, st * 128:(st + 1) * 128],
                                 wv_sb[:, c, cs],
                                 start=(c == 0), stop=(c == NDC - 1))
            for hh in range(2):
                h = 2 * h01 + hh
                nc.vector.tensor_copy(out=v_sb[h][:, st, 0:DH],
                                      in_=ps[:, hh * DH:(hh + 1) * DH])

        def pair_stats(t):
            # bn_aggr + cross-partition combine + Newton rsqrt -> (M', r)
            h0, h1 = 2 * t, 2 * t + 1
            scg = pd.tile([1, 2, 3], f32, tag=f"scg{t}", name=f"scg{t}")
            for i, h in enumerate((h0, h1)):
                mvh = pd.tile([64, 2], f32, tag="mv", bufs=2, name=f"mv{h}")
                nc.vector.bn_aggr(out=mvh, in_=bnst[h])
                stk = stk_all[h]
                nc.vector.tensor_add(stk[:, 0:1], mvh[:, 0:1], bvh_sb[:, h:h + 1])
                nc.vector.tensor_copy(stk[:, 1:2], mvh[:, 1:2])
                nc.vector.tensor_mul(stk[:, 2:3], stk[:, 0:1], stk[:, 0:1])
                stp = ppp.tile([1, 3], f32, tag="pp", name=f"stp{h}")
                nc.tensor.matmul(stp, ones64, stk, start=True, stop=True)
                nc.vector.tensor_scalar(out=scg[:, i, :], in0=stp,
                                        scalar1=1.0 / 64.0, scalar2=None,
                                        op0=ALU.mult)
            # var_tot = E[var] + E[(m+bv)^2] - M'^2 ; r = rsqrt(var_tot + eps)
            m2 = pd.tile([1, 2], f32, tag=f"m2{t}", name=f"m2{t}")
            nc.vector.tensor_mul(m2, scg[:, :, 0], scg[:, :, 0])
            vr = pd.tile([1, 2], f32, tag=f"vr{t}", name=f"vr{t}")
            nc.vector.tensor_add(vr, scg[:, :, 1], scg[:, :, 2])
            nc.vector.tensor_tensor(out=vr, in0=vr, in1=m2, op=ALU.subtract)
            nc.vector.tensor_scalar(out=vr, in0=vr, scalar1=EPS, scalar2=None,
                                    op0=ALU.add)
            yr = pd.tile([1, 2], f32, tag=f"yr{t}", name=f"yr{t}")
            ish = pd.tile([1, 2], i32, tag=f"ish{t}", name=f"ish{t}")
            nc.vector.tensor_scalar(out=ish, in0=vr[:, :].bitcast(i32),
                                    scalar1=1, scalar2=None,
                                    op0=ALU.logical_shift_right)
            nc.vector.tensor_tensor(
                out=yr[:, :].bitcast(i32), in0=magic_sb[:, :].bitcast(i32),
                in1=ish, op=ALU.subtract)
            tt = pd.tile([1, 2], f32, tag=f"tt{t}", name=f"tt{t}")
            for _ in range(3):
                nc.vector.tensor_mul(tt, yr, yr)
                nc.vector.tensor_mul(tt, tt, vr)
                nc.vector.tensor_scalar(out=tt, in0=tt, scalar1=-0.5,
                                        scalar2=1.5, op0=ALU.mult, op1=ALU.add)
                nc.vector.tensor_mul(yr, yr, tt)
            for i, h in enumerate((h0, h1)):
                mr = pd.tile([1, 2], f32, tag="mr", bufs=4, name=f"mr{h}")
                nc.vector.tensor_copy(mr[:, 0:1], scg[:, i, 0:1])
                nc.vector.tensor_copy(mr[:, 1:2], yr[:, i:i + 1])
                mr_sb[h] = mr
            # payload [M_h0, r_h0, M_h1, r_h1]
            msc = pd.tile([1, 4], f32, tag=f"msc{t}", name=f"msc{t}")
            for i, h in enumerate((h0, h1)):
                nc.vector.tensor_copy(msc[:, 2 * i:2 * i + 1], mr_sb[h][:, 0:1])
                nc.vector.tensor_copy(msc[:, 2 * i + 1:2 * i + 2],
                                      mr_sb[h][:, 1:2])
            state[f"msc{t}"] = msc

        def gather(in_t, out_t):
            if with_collective:
                nc.gpsimd.collective_compute(
                    "AllGather", ALU.bypass, replica_groups=RG,
                    ins=[in_t[:].opt()], outs=[out_t[:].opt()])
            else:
                for g in range(4):
                    nc.sync.dma_start(out=out_t[g], in_=in_t[:, :])

        def stage_chunk(t, q):
            # gathered chunk -> nrmg[:, 2g+t, q-slice]; then payload extract
            nc.gpsimd.dma_start(
                out=nrmg[:, t::2, q * QT:(q + 1) * QT],
                in_=agc_out[t][q][:, :, 0:QT].rearrange("g p q -> p g q"))
            if t == 0 and q == 3:
                sc = pd.tile([1, 4, SCC], bf16, tag="sc16a", name="sc16a")
                nc.gpsimd.dma_start(
                    out=sc,
                    in_=agc_out[0][3][:, 0:1, QT:QT + SCC].rearrange("g p c -> p g c"))
                state["sc16a"] = sc

        qk1_work = []
        for st in range(NQT):
            qk1_work.append(("q", st))
            qk1_work.append(("k", st))

        def attn_qt(t, qt):
            h0, h1 = 2 * t, 2 * t + 1
            av0 = pav.tile([DH + 1, QT], f32, tag="av", name=f"av{t}{qt}a")
            av1 = pav.tile([DH + 1, QT], f32, tag="av", name=f"av{t}{qt}b")
            for kt in range(NKT):
                sps = psc.tile([128, 2 * QT], f32, tag="s", name=f"s{t}{qt}{kt}")
                for o in range(2):
                    nc.tensor.matmul(
                        sps[:, o * QT:(o + 1) * QT],
                        kT_sb[64 * o:64 * (o + 1), t, kt * 128:(kt + 1) * 128],
                        qT_sb[64 * o:64 * (o + 1), t, qt * QT:(qt + 1) * QT],
                        start=True, stop=True)
                e_sb = pexp.tile([128, 2 * QT], bf16, tag="e", name=f"e{t}{qt}{kt}")
                nc.scalar.activation(e_sb, sps, AF.Exp)
                nc.tensor.matmul(av0, v_sb[h0][:, kt, :], e_sb[:, 0:QT],
                                 start=(kt == 0), stop=(kt == NKT - 1))
                nc.tensor.matmul(av1, v_sb[h1][:, kt, :], e_sb[:, QT:2 * QT],
                                 start=(kt == 0), stop=(kt == NKT - 1))
                if t == 0:
                    # sprinkle pair-1 projections into the exp-bound loop
                    if kt % 4 == 3:
                        v_proj_st(qt * 4 + (kt - 3) // 4, 1)
                    elif kt % 8 == 5 and qk1_work:
                        kind, pst = qk1_work.pop(0)
                        if kind == "q":
                            qk_proj_st(1, pst, wq_sb, bqp_sb, qT_sb)
                        else:
                            qk_proj_st(1, pst, wk_sb, bkp_sb, kT_sb)
            # per-qt softmax normalization (overlaps next qt's attention)
            for i, (h, av) in enumerate(((h0, av0), (h1, av1))):
                row = 2 * (qt % 2) + i
                zt = prb.tile([DH + 1, QT], f32, tag="zt", name=f"zt{t}{qt}{i}")
                nc.vector.tensor_copy(out=zt, in_=av)
                nc.sync.dma_start(out=rb_d[row:row + 1, :],
                                  in_=zt[DH:DH + 1, :])
                rb = prb.tile([64, QT], f32, tag="rb", name=f"rb{t}{qt}{i}")
                nc.sync.dma_start(out=rb,
                                  in_=rb_d[row:row + 1, :].to_broadcast([64, QT]))
                nc.vector.reciprocal_approx_fast(rb, rb)
                zsl = z_sb[h][:, qt * QT:(qt + 1) * QT]
                nc.vector.tensor_mul(zsl, zt[0:DH, :], rb)
                nc.vector.bn_stats(out=bnst[h][:, qt, :], in_=zsl)
            # gather this qt's raw z chunk straight from the z tiles
            if qt == 3:
                pair_stats(t)
                if t == 0:
                    nc.sync.dma_start(
                        out=agc_in[0][3][0:1, QT:QT + SCC],
                        in_=state["msc0"][0:1, :].bitcast(bf16))
                else:
                    # pair-1 scalars: tiny gather fired before the last chunk
                    sc1sb = pd.tile([1, SCC], bf16, tag="sc1sb", name="sc1sb")
                    nc.vector.tensor_copy(out=sc1sb,
                                          in_=state["msc1"][0:1, :].bitcast(bf16))
                    nc.sync.dma_start(out=sc1_in[:, :], in_=sc1sb)
                    gather(sc1_in, sc1_out)
                    scb = pd.tile([1, 4, SCC], bf16, tag="sc16b", name="sc16b")
                    nc.gpsimd.dma_start(
                        out=scb,
                        in_=sc1_out[:, :, :].rearrange("g p c -> p g c"))
                    state["sc16b"] = scb
            nc.sync.dma_start(out=agc_in[t][qt][0:64, 0:QT],
                              in_=z_sb[h0][:, qt * QT:(qt + 1) * QT])
            nc.sync.dma_start(out=agc_in[t][qt][64:128, 0:QT],
                              in_=z_sb[h1][:, qt * QT:(qt + 1) * QT])
            gather(agc_in[t][qt], agc_out[t][qt])
            stage_chunk(t, qt)

        # ---- lead: q/k for pair 0 (c-outer), v for heads 0-1 ----
        for st0 in (0, 2):
            qk_proj_pair(0, st0, wq_sb, bqp_sb, qT_sb)
            qk_proj_pair(0, st0, wk_sb, bkp_sb, kT_sb)
        for st in range(NKT):
            v_proj_st(st, 0)
        for h in range(2):
            nc.vector.memset(v_sb[h][:, :, DH:DH + 1], 1.0)

        # ---- pair 0 attention (v1 + qk1 sprinkled) ----
        for qt in range(NQT):
            attn_qt(0, qt)
        for h in range(2, 4):
            nc.vector.memset(v_sb[h][:, :, DH:DH + 1], 1.0)

        # receiver maps for pair-0 chunks (payload lands mid-pair-1)
        def build_maps(t, sc, pool, ptag):
            scf = sc[:, :, :].bitcast(f32)     # [1, 4, 4]: M0 r0 M1 r1
            mm = pool.tile([128, 4], f32, tag=ptag, name=f"mapm{t}")
            rr = pool.tile([128, 4], f32, tag=ptag, name=f"mapr{t}")
            for o in range(2):
                nc.tensor.matmul(mm[64 * o:64 * (o + 1), :], ones1r,
                                 scf[:, :, 2 * o], start=True, stop=True)
                nc.tensor.matmul(rr[64 * o:64 * (o + 1), :], ones1r,
                                 scf[:, :, 2 * o + 1], start=True, stop=True)
            rmap = pg.tile([128, 4], f32, tag=f"rmap{t}")
            nc.vector.tensor_copy(rmap, rr)
            mvec = pg.tile([128, 4], bf16, tag=f"mvec{t}")
            mtmp = pg.tile([128, 4], f32, tag=f"mtmp{t}")
            nc.vector.tensor_tensor(out=mtmp, in0=bvo_sb[:, t::2], in1=mm,
                                    op=ALU.subtract)
            nc.vector.tensor_mul(mvec, mtmp, rmap)
            wos = pg.tile([128, 4, CW], bf16, tag=f"wos{t}")
            for g in range(4):
                nc.vector.tensor_scalar(out=wos[:, g, :],
                                        in0=wo_sb[:, 2 * g + t, :],
                                        scalar1=rmap[:, g:g + 1], scalar2=None,
                                        op0=ALU.mult)
            return mvec, wos

        # ---- pair 1 attention with continued chunk gathers ----
        pg = ctx.enter_context(tc.tile_pool(name="pg", bufs=1))
        attn_qt(1, 0)
        attn_qt(1, 1)
        attn_qt(1, 2)
        mvec0, wos0 = build_maps(0, state["sc16a"], ppp, "pp")
        attn_qt(1, 3)

        if debug:
            for h in range(HPC):
                nc.gpsimd.dma_start(out=dbgz_d[h], in_=z_sb[h])
                nc.sync.dma_start(out=dbgmr_d[h:h + 1, :], in_=mr_sb[h])
            nc.sync.dma_start(out=dbgnr_d[0], in_=nrmg[:, 0::2, :])
            nc.sync.dma_start(out=dbgnr_d[1], in_=nrmg[:, 1::2, :])

        # ---- tail: pair-1 maps, bias row, out-projection ----
        with tc.tile_pool(name="pystage", bufs=1) as pystage:
            ystage = [pystage.tile([128, S], f32, tag=f"ys{nt}", name=f"ys{nt}")
                      for nt in range(2)]
            # pair-0 accumulation runs while the last gather is in flight
            yp0 = [psc.tile([128, 2 * QT], f32, tag="s", name=f"yp0_{j}")
                   for j in range(2)]
            for g in range(4):
                for st in range(NQT):
                    nc.tensor.matmul(
                        yp0[st // 2][:, (st % 2) * QT:(st % 2 + 1) * QT],
                        wos0[:, g, 0:128],
                        nrmg[:, 2 * g, st * QT:(st + 1) * QT],
                        start=(g == 0), stop=False)
            yp1a = [ppp.tile([128, QT], f32, tag="pp", name=f"yp1a_{st}")
                    for st in range(2)]
            for g in range(4):
                for st in range(2):
                    nc.tensor.matmul(
                        yp1a[st],
                        wos0[:, g, 128:256],
                        nrmg[:, 2 * g, st * QT:(st + 1) * QT],
                        start=(g == 0), stop=False)

            mvec1, wos1 = build_maps(1, state["sc16b"], pav, "av")

            # bias row: bo + sum_d (bv-M)*r*wo over all chunks
            cstp = pav.tile([1, CW], f32, tag="av", name="cstp")
            for t, mv in ((0, mvec0), (1, mvec1)):
                for g in range(4):
                    nc.tensor.matmul(cstp, mv[:, g:g + 1], wo_sb[:, 2 * g + t, :],
                                     start=(t == 0 and g == 0),
                                     stop=(t == 1 and g == 3))
            brow = pg.tile([1, CW], bf16, tag="brow")
            nc.vector.tensor_tensor(out=brow, in0=bor_sb, in1=cstp, op=ALU.add)

            # nt0: pair-1 chunks + bias + drain (st3's chunk lands last)
            for g in range(4):
                for st in range(NQT):
                    nc.tensor.matmul(
                        yp0[st // 2][:, (st % 2) * QT:(st % 2 + 1) * QT],
                        wos1[:, g, 0:128],
                        nrmg[:, 2 * g + 1, st * QT:(st + 1) * QT],
                        start=False, stop=False)
            for st in range(NQT):
                src = yp0[st // 2][:, (st % 2) * QT:(st % 2 + 1) * QT]
                nc.tensor.matmul(src, brow[:, 0:128], onesrow,
                                 start=False, stop=True)
                dst = ystage[0][:, st * QT:(st + 1) * QT]
                if st % 2 == 0:
                    nc.scalar.activation(dst, src, AF.Copy)
                else:
                    nc.vector.tensor_copy(out=dst, in_=src)
                nc.sync.dma_start(out=y_d[0, :, st * QT:(st + 1) * QT],
                                  in_=ystage[0][:, st * QT:(st + 1) * QT])

            # nt1: st0-1 in pp psum; st2-3 reuse sc psum after nt0 drains
            for g in range(4):
                for st in range(2):
                    nc.tensor.matmul(yp1a[st], wos1[:, g, 128:256],
                                     nrmg[:, 2 * g + 1, st * QT:(st + 1) * QT],
                                     start=False, stop=False)
            for st in range(2):
                nc.tensor.matmul(yp1a[st], brow[:, 128:256], onesrow,
                                 start=False, stop=True)
            yp1b = psc.tile([128, 2 * QT], f32, tag="s", name="yp1b")
            for g in range(4):
                for st in range(2, NQT):
                    nc.tensor.matmul(
                        yp1b[:, (st - 2) * QT:(st - 1) * QT],
                        wos0[:, g, 128:256],
                        nrmg[:, 2 * g, st * QT:(st + 1) * QT],
                        start=(g == 0), stop=False)
            for g in range(4):
                for st in range(2, NQT):
                    nc.tensor.matmul(
                        yp1b[:, (st - 2) * QT:(st - 1) * QT],
                        wos1[:, g, 128:256],
                        nrmg[:, 2 * g + 1, st * QT:(st + 1) * QT],
                        start=False, stop=False)
            for st in range(2, NQT):
                nc.tensor.matmul(yp1b[:, (st - 2) * QT:(st - 1) * QT],
                                 brow[:, 128:256], onesrow,
                                 start=False, stop=True)
            for st in range(NQT):
                src = (yp1a[st] if st < 2
                       else yp1b[:, (st - 2) * QT:(st - 1) * QT])
                dst = ystage[1][:, st * QT:(st + 1) * QT]
                if st % 2 == 0:
                    nc.scalar.activation(dst, src, AF.Copy)
                else:
                    nc.vector.tensor_copy(out=dst, in_=src)
                nc.sync.dma_start(out=y_d[1, :, st * QT:(st + 1) * QT],
                                  in_=ystage[1][:, st * QT:(st + 1) * QT])

    nc.compile()
    return nc


def _get_nc():
    if "nc" not in _cache:
        _cache["nc"] = _build()
    return _cache["nc"]


def _host_prep(x, Wq, bq, Wk, bk, Wv, bv, Wo, bo, lq1, lk1, lq2, lk2, gn_w, gn_b):
    x = np.asarray(x, np.float32)
    lam = (np.exp((np.asarray(lq1) * np.asarray(lk1)).sum(-1))
           - np.exp((np.asarray(lq2) * np.asarray(lk2)).sum(-1)) + LAMBDA_INIT)
    qscale = (DH ** -0.5) * lam
    Wq_eff = (np.asarray(Wq).reshape(D, H, DH) * qscale[None, :, None]).reshape(D, D)
    bq_eff = (np.asarray(bq).reshape(H, DH) * qscale[:, None]).reshape(D)
    gw = np.asarray(gn_w).reshape(D)
    gb = np.asarray(gn_b).reshape(D)
    Wo_eff = np.asarray(Wo) * gw[:, None]
    bo_eff = np.asarray(bo) + gb @ np.asarray(Wo)
    bk_full = np.asarray(bk)
    bv_full = np.asarray(bv, np.float32)

    # Gathered-row order (chunk (g,t), partition (o,dh) -> head 4g+2t+o) is
    # exactly the original row-major head order, so Wo_eff rows need no
    # permutation.
    xT = np.ascontiguousarray(x.transpose(0, 2, 1))  # [B, D, S]
    bf = ml_dtypes.bfloat16

    def pair_partition_layout(vec256):
        # [256] (head-major: (2t+o)*64+dh) -> [128, 2] with row o*64+dh, col t
        return np.ascontiguousarray(
            vec256.reshape(2, 2, DH).transpose(1, 2, 0).reshape(128, 2)
        ).astype(np.float32)

    # receiver bv map: bvo[o*64+dh, 2g+t] = bv[(4g+2t+o)*64+dh]
    bvo = np.ascontiguousarray(
        bv_full.reshape(4, 2, 2, DH).transpose(2, 3, 0, 1).reshape(128, 8)
    ).astype(np.float32)

    in_maps = []
    for c in range(N_CORES):
        b, hg = c // 4, c % 4
        cs = slice(CW * hg, CW * (hg + 1))
        in_maps.append({
            "xt": np.ascontiguousarray(xT[b]).astype(bf),
            "wq": np.ascontiguousarray(Wq_eff[:, cs]).astype(bf),
            "wk": np.ascontiguousarray(np.asarray(Wk)[:, cs]).astype(bf),
            "wv": np.ascontiguousarray(np.asarray(Wv)[:, cs]).astype(bf),
            "wo": np.ascontiguousarray(Wo_eff[:, cs]).astype(bf),
            "bqp": pair_partition_layout(bq_eff[cs]),
            "bkp": pair_partition_layout(bk_full[cs]),
            "bvh": np.ascontiguousarray(
                bv_full[cs].reshape(HPC, DH).T).astype(np.float32),
            "bvo": bvo,
            "bo": np.ascontiguousarray(bo_eff[cs]).astype(bf),
        })
    return in_maps


def _host_gather(outs):
    # core c=4b+hg produced output columns [256*hg, 256*(hg+1)) as [2,128,S]
    yT = np.empty((B, D, S), np.float32)
    for b in range(B):
        for hg in range(4):
            q = np.asarray(outs[4 * b + hg]["y"]).reshape(CW, S)
            yT[b, CW * hg:CW * (hg + 1), :] = q
    return np.ascontiguousarray(yT.transpose(0, 2, 1))


def kernel(x, Wq, bq, Wk, bk, Wv, bv, Wo, bo, lq1, lk1, lq2, lk2, gn_w, gn_b):
    from concourse.bass_utils import run_bass_kernel_spmd

    in_maps = _host_prep(x, Wq, bq, Wk, bk, Wv, bv, Wo, bo,
                         lq1, lk1, lq2, lk2, gn_w, gn_b)
    nc = _get_nc()
    res = run_bass_kernel_spmd(nc, in_maps, core_ids=list(range(N_CORES)))
    return _host_gather(res.results)


# revision 29
# speedup vs baseline: 1.0889x; 1.0141x over previous
"""Multi-head differential attention on 8 Trainium2 NeuronCores.

Sharding: core c -> batch c//4, head-group c%4 (4 of 16 heads).
Per core: QKV projection for its heads (pair-1 q/k and v sprinkled into
pair-0's exp-bound attention loop), k-major attention (scores via
row-group-packed 64-partition matmul pairs; softmax denominators from a
ones-row appended to V), per-(batch,head,qt-chunk) softmax normalization
pipelined inside the attention loop, GroupNorm statistics via bn_stats
with a DVE Newton rsqrt (no scalar-engine table switch).

Raw (pre-GroupNorm) z is AllGathered in eight per-qt [128,512] chunks
pipelined across both pairs' attention so the slow collective fabric
streams continuously.  Each pair's GN scalars (mean/rstd per head)
travel as bitcast payload: pair 0's ride its last chunk, pair 1's go in
a tiny dedicated gather fired before the last chunk so the Wo-scaling
fold overlaps the final transfer.  The receiver folds (bv-M)*r into a
scaled Wo and a bias row; the out-projection accumulates pair-0 chunks
while the last gather is in flight and finishes per seq-tile.

Host side folds: lambda and softmax scale into Wq/bq; GroupNorm affine
into Wo/bo.  x is pre-transposed per batch and cast to bf16.
"""

import numpy as np
import ml_dtypes

B, S, D, H, DH = 2, 2048, 1024, 16, 64
HPC = 4            # heads per core
CW = HPC * DH      # attention columns per core (256)
EPS = 1e-5
LAMBDA_INIT = 0.8
N_CORES = 8
SCC = 8            # scalar payload columns (4 f32 as 8 bf16)
RSQRT_MAGIC = 1.32118221e+19   # f32 with bits 0x5f3759df

NQT = 4            # query tiles of 512
QT = 512
NKT = 16           # key tiles of 128
NDC = 8            # d-chunks of 128
RG = [[0, 1, 2, 3], [4, 5, 6, 7]]

_cache = {}


def _build(with_collective=True, debug=False):
    from contextlib import ExitStack
    import concourse.bass as bass
    from concourse import bacc
    import concourse.tile as tile
    import concourse.mybir as mybir

    f32 = mybir.dt.float32
    i32 = mybir.dt.int32
    bf16 = mybir.dt.bfloat16
    AF = mybir.ActivationFunctionType
    ALU = mybir.AluOpType

    nc = bacc.Bacc("TRN2", target_bir_lowering=False, debug=False,
                   num_devices=N_CORES)

    xt_d = nc.dram_tensor("xt", [D, S], bf16, kind="ExternalInput")
    wq_d = nc.dram_tensor("wq", [D, CW], bf16, kind="ExternalInput")
    wk_d = nc.dram_tensor("wk", [D, CW], bf16, kind="ExternalInput")
    wv_d = nc.dram_tensor("wv", [D, CW], bf16, kind="ExternalInput")
    # wo: gathered-row layout [(g t o p), quarter-cols]
    wo_d = nc.dram_tensor("wo", [D, CW], bf16, kind="ExternalInput")
    bqp_d = nc.dram_tensor("bqp", [128, 2], f32, kind="ExternalInput")
    bkp_d = nc.dram_tensor("bkp", [128, 2], f32, kind="ExternalInput")
    bvh_d = nc.dram_tensor("bvh", [64, HPC], f32, kind="ExternalInput")
    bvo_d = nc.dram_tensor("bvo", [128, 2 * HPC], f32, kind="ExternalInput")
    bo_d = nc.dram_tensor("bo", [CW], bf16, kind="ExternalInput")
    y_d = nc.dram_tensor("y", [2, 128, S], f32, kind="ExternalOutput")

    # per-(pair,qt) chunk gathers; qt3 chunks carry the payload columns
    def chunk_w(t, q):
        return QT + (SCC if q == 3 else 0)
    agc_in = [[nc.dram_tensor(f"agc_in{t}{q}", [128, chunk_w(t, q)], bf16)
               for q in range(NQT)] for t in range(2)]
    agc_out = [[nc.dram_tensor(f"agc_out{t}{q}", [4, 128, chunk_w(t, q)], bf16)
                for q in range(NQT)] for t in range(2)]
    rb_d = nc.dram_tensor("rb_bounce", [4, QT], f32)
    if debug:
        dbgz_d = nc.dram_tensor("dbgz", [HPC, DH, S], f32, kind="ExternalOutput")
        dbgmr_d = nc.dram_tensor("dbgmr", [HPC, 2], f32, kind="ExternalOutput")
        dbgnr_d = nc.dram_tensor("dbgnr", [2, 128, 4, S], bf16, kind="ExternalOutput")

    with ExitStack() as ctx:
        tc = ctx.enter_context(tile.TileContext(nc))
        const = ctx.enter_context(tc.tile_pool(name="const", bufs=1))
        big = ctx.enter_context(tc.tile_pool(name="big", bufs=1))
        psc = ctx.enter_context(tc.tile_pool(name="psc", bufs=2, space="PSUM"))
        pav = ctx.enter_context(tc.tile_pool(name="pav", bufs=2, space="PSUM"))
        ppp = ctx.enter_context(tc.tile_pool(name="ppp", bufs=2, space="PSUM"))
        pexp = ctx.enter_context(tc.tile_pool(name="pexp", bufs=4))
        pd = ctx.enter_context(tc.tile_pool(name="pd", bufs=1))
        prb = ctx.enter_context(tc.tile_pool(name="prb", bufs=2))

        # ---- input DMAs: priority order on the sync queue ----
        wq_sb = const.tile([128, NDC, CW], bf16, tag="wq")
        nc.sync.dma_start(out=wq_sb, in_=wq_d[:, :].rearrange("(c p) n -> p c n", p=128))
        pxt = ctx.enter_context(tc.tile_pool(name="pxt", bufs=1))
        xt_sb = [pxt.tile([128, S], bf16, tag=f"xt{c}", name=f"xt{c}")
                 for c in range(NDC)]
        for c in range(NDC):
            nc.sync.dma_start(out=xt_sb[c], in_=xt_d[c * 128:(c + 1) * 128, :])
        wk_sb = const.tile([128, NDC, CW], bf16, tag="wk")
        wv_sb = const.tile([128, NDC, CW], bf16, tag="wv")
        nc.sync.dma_start(out=wk_sb, in_=wk_d[:, :].rearrange("(c p) n -> p c n", p=128))
        nc.sync.dma_start(out=wv_sb, in_=wv_d[:, :].rearrange("(c p) n -> p c n", p=128))
        wo_sb = const.tile([128, NDC, CW], bf16, tag="wo")
        nc.sync.dma_start(out=wo_sb, in_=wo_d[:, :].rearrange("(c p) n -> p c n", p=128))

        # small constants on the gpsimd queue
        bqp_sb = const.tile([128, 2], f32, tag="bqp")
        bkp_sb = const.tile([128, 2], f32, tag="bkp")
        bvh_sb = const.tile([64, HPC], f32, tag="bvh")
        bvo_sb = const.tile([128, 2 * HPC], f32, tag="bvo")
        bor_sb = const.tile([1, CW], bf16, tag="bor")
        nc.gpsimd.dma_start(out=bqp_sb, in_=bqp_d[:, :])
        nc.gpsimd.dma_start(out=bkp_sb, in_=bkp_d[:, :])
        nc.gpsimd.dma_start(out=bvh_sb, in_=bvh_d[:, :])
        nc.gpsimd.dma_start(out=bvo_sb, in_=bvo_d[:, :])
        nc.gpsimd.dma_start(out=bor_sb, in_=bo_d[:].rearrange("(a n) -> a n", a=1))

        ones64 = const.tile([64, 1], f32, tag="ones64")
        nc.vector.memset(ones64, 1.0)
        ones1r = const.tile([1, 64], f32, tag="ones1r")
        nc.vector.memset(ones1r, 1.0)
        onesrow = const.tile([1, QT], bf16, tag="onesrow")
        nc.vector.memset(onesrow, 1.0)
        magic_sb = const.tile([1, 2], f32, tag="magic")
        nc.vector.memset(magic_sb, RSQRT_MAGIC)

        # pre-warm the exp activation table while DMAs stream
        warm_sb = const.tile([1, 1], f32, tag="warm")
        nc.vector.memset(warm_sb, 0.0)
        warm2 = const.tile([1, 1], f32, tag="warm2")
        nc.scalar.activation(warm2, warm_sb, AF.Exp)

        qT_sb = big.tile([128, 2, S], bf16, tag="qT")   # pair t; head-parity o rows
        kT_sb = big.tile([128, 2, S], bf16, tag="kT")
        v_sb = [big.tile([128, NKT, DH + 1], bf16, tag=f"v{h}", name=f"v{h}")
                for h in range(HPC)]
        z_sb = [big.tile([DH, S], bf16, tag=f"z{h}", name=f"z{h}")
                for h in range(HPC)]
        nrmg = big.tile([128, NDC, S], bf16, tag="nrmg")   # chunk c=2g+t

        bnst = [pd.tile([64, NQT, 6], f32, tag=f"bn{h}", name=f"bnst{h}")
                for h in range(HPC)]
        stk_all = [pd.tile([64, 3], f32, tag=f"stk{h}", name=f"stk{h}")
                   for h in range(HPC)]
        mr_sb = {}
        state = {}

        def qk_proj_pair(t, st0, w_sb, bp_sb, dst):
            # two query-tile columns, c-outer so work starts on xt chunk 0
            ps = [ppp.tile([128, QT], f32, tag="pp",
                           name=f"qk{t}{st0}{j}{w_sb.tensor.name}")
                  for j in range(2)]
            for c in range(NDC):
                for j in range(2):
                    nc.tensor.matmul(ps[j], w_sb[:, c, t * 128:(t + 1) * 128],
                                     xt_sb[c][:, (st0 + j) * QT:(st0 + j + 1) * QT],
                                     start=(c == 0), stop=(c == NDC - 1))
            for j in range(2):
                nc.vector.tensor_scalar(
                    out=dst[:, t, (st0 + j) * QT:(st0 + j + 1) * QT],
                    in0=ps[j], scalar1=bp_sb[:, t:t + 1],
                    scalar2=None, op0=ALU.add)

        def qk_proj_st(t, st, w_sb, bp_sb, dst):
            ps = ppp.tile([128, QT], f32, tag="pp",
                          name=f"qk{t}{st}{w_sb.tensor.name}")
            for c in range(NDC):
                nc.tensor.matmul(ps, w_sb[:, c, t * 128:(t + 1) * 128],
                                 xt_sb[c][:, st * QT:(st + 1) * QT],
                                 start=(c == 0), stop=(c == NDC - 1))
            nc.vector.tensor_scalar(out=dst[:, t, st * QT:(st + 1) * QT],
                                    in0=ps, scalar1=bp_sb[:, t:t + 1],
                                    scalar2=None, op0=ALU.add)

        def v_proj_st(st, h01):
            # v for heads [2*h01, 2*h01+1] at key-tile st
            cs = slice(h01 * 2 * DH, (h01 + 1) * 2 * DH)
            ps = ppp.tile([128, QT], f32, tag="pp", name=f"v{h01}{st}")
            for c in range(NDC):
                nc.tensor.matmul(ps[:, 0:2 * DH],
                                 xt_sb[c][:, st * 128:(st + 1) * 128],
                                 wv_sb[:, c, cs],
                                 start=(c == 0), stop=(c == NDC - 1))
            for hh in range(2):
                h = 2 * h01 + hh
                nc.vector.tensor_copy(out=v_sb[h][:, st, 0:DH],
                                      in_=ps[:, hh * DH:(hh + 1) * DH])

        def pair_stats(t):
            # bn_aggr + cross-partition combine + Newton rsqrt -> (M', r)
            h0, h1 = 2 * t, 2 * t + 1
            scg = pd.tile([1, 2, 3], f32, tag=f"scg{t}", name=f"scg{t}")
            for i, h in enumerate((h0, h1)):
                mvh = pd.tile([64, 2], f32, tag="mv", bufs=2, name=f"mv{h}")
                nc.vector.bn_aggr(out=mvh, in_=bnst[h])
                stk = stk_all[h]
                nc.vector.tensor_add(stk[:, 0:1], mvh[:, 0:1], bvh_sb[:, h:h + 1])
                nc.vector.tensor_copy(stk[:, 1:2], mvh[:, 1:2])
                nc.vector.tensor_mul(stk[:, 2:3], stk[:, 0:1], stk[:, 0:1])
                stp = ppp.tile([1, 3], f32, tag="pp", name=f"stp{h}")
                nc.tensor.matmul(stp, ones64, stk, start=True, stop=True)
                nc.vector.tensor_scalar(out=scg[:, i, :], in0=stp,
                                        scalar1=1.0 / 64.0, scalar2=None,
                                        op0=ALU.mult)
            # var_tot = E[var] + E[(m+bv)^2] - M'^2 ; r = rsqrt(var_tot + eps)
            m2 = pd.tile([1, 2], f32, tag=f"m2{t}", name=f"m2{t}")
            nc.vector.tensor_mul(m2, scg[:, :, 0], scg[:, :, 0])
            vr = pd.tile([1, 2], f32, tag=f"vr{t}", name=f"vr{t}")
            nc.vector.tensor_add(vr, scg[:, :, 1], scg[:, :, 2])
            nc.vector.tensor_tensor(out=vr, in0=vr, in1=m2, op=ALU.subtract)
            nc.vector.tensor_scalar(out=vr, in0=vr, scalar1=EPS, scalar2=None,
                                    op0=ALU.add)
            yr = pd.tile([1, 2], f32, tag=f"yr{t}", name=f"yr{t}")
            ish = pd.tile([1, 2], i32, tag=f"ish{t}", name=f"ish{t}")
            nc.vector.tensor_scalar(out=ish, in0=vr[:, :].bitcast(i32),
                                    scalar1=1, scalar2=None,
                                    op0=ALU.logical_shift_right)
            nc.vector.tensor_tensor(
                out=yr[:, :].bitcast(i32), in0=magic_sb[:, :].bitcast(i32),
                in1=ish, op=ALU.subtract)
            tt = pd.tile([1, 2], f32, tag=f"tt{t}", name=f"tt{t}")
            for _ in range(3):
                nc.vector.tensor_mul(tt, yr, yr)
                nc.vector.tensor_mul(tt, tt, vr)
                nc.vector.tensor_scalar(out=tt, in0=tt, scalar1=-0.5,
                                        scalar2=1.5, op0=ALU.mult, op1=ALU.add)
                nc.vector.tensor_mul(yr, yr, tt)
            for i, h in enumerate((h0, h1)):
                mr = pd.tile([1, 2], f32, tag="mr", bufs=4, name=f"mr{h}")
                nc.vector.tensor_copy(mr[:, 0:1], scg[:, i, 0:1])
                nc.vector.tensor_copy(mr[:, 1:2], yr[:, i:i + 1])
                mr_sb[h] = mr
            # payload [M_h0, r_h0, M_h1, r_h1]
            msc = pd.tile([1, 4], f32, tag=f"msc{t}", name=f"msc{t}")
            for i, h in enumerate((h0, h1)):
                nc.vector.tensor_copy(msc[:, 2 * i:2 * i + 1], mr_sb[h][:, 0:1])
                nc.vector.tensor_copy(msc[:, 2 * i + 1:2 * i + 2],
                                      mr_sb[h][:, 1:2])
            state[f"msc{t}"] = msc

        def gather(in_t, out_t):
            if with_collective:
                nc.gpsimd.collective_compute(
                    "AllGather", ALU.bypass, replica_groups=RG,
                    ins=[in_t[:].opt()], outs=[out_t[:].opt()])
            else:
                for g in range(4):
                    nc.sync.dma_start(out=out_t[g], in_=in_t[:, :])

        def stage_chunk(t, q):
            # gathered chunk -> nrmg[:, 2g+t, q-slice]; then payload extract
            nc.gpsimd.dma_start(
                out=nrmg[:, t::2, q * QT:(q + 1) * QT],
                in_=agc_out[t][q][:, :, 0:QT].rearrange("g p q -> p g q"))
            if q == 3:
                sc = pd.tile([1, 4, SCC], bf16, tag=f"sc16{t}", name=f"sc16{t}")
                nc.gpsimd.dma_start(
                    out=sc,
                    in_=agc_out[t][3][:, 0:1, QT:QT + SCC].rearrange("g p c -> p g c"))
                state[f"sc16{t}"] = sc

        qk1_work = []
        for st in range(NQT):
            qk1_work.append(("q", st))
            qk1_work.append(("k", st))

        SEQA = [(t, qt, kt) for t in (0, 1) for qt in range(NQT)
                for kt in range(NKT)]
        e_tiles = {}
        av_cur = {}

        def emit_scores_exp(j):
            t, qt, kt = SEQA[j]
            sps = psc.tile([128, 2 * QT], f32, tag="s", name=f"s{t}{qt}{kt}")
            for o in range(2):
                nc.tensor.matmul(
                    sps[:, o * QT:(o + 1) * QT],
                    kT_sb[64 * o:64 * (o + 1), t, kt * 128:(kt + 1) * 128],
                    qT_sb[64 * o:64 * (o + 1), t, qt * QT:(qt + 1) * QT],
                    start=True, stop=True)
            e_sb = pexp.tile([128, 2 * QT], bf16, tag="e", name=f"e{t}{qt}{kt}")
            nc.scalar.activation(e_sb, sps, AF.Exp)
            e_tiles[j] = e_sb

        def attn_iter(j):
            t, qt, kt = SEQA[j]
            h0, h1 = 2 * t, 2 * t + 1
            if j + 1 < len(SEQA):
                emit_scores_exp(j + 1)
            if kt == 0:
                av_cur[0] = pav.tile([DH + 1, QT], f32, tag="av", name=f"av{t}{qt}a")
                av_cur[1] = pav.tile([DH + 1, QT], f32, tag="av", name=f"av{t}{qt}b")
            av0, av1 = av_cur[0], av_cur[1]
            e_sb = e_tiles.pop(j)
            nc.tensor.matmul(av0, v_sb[h0][:, kt, :], e_sb[:, 0:QT],
                             start=(kt == 0), stop=(kt == NKT - 1))
            nc.tensor.matmul(av1, v_sb[h1][:, kt, :], e_sb[:, QT:2 * QT],
                             start=(kt == 0), stop=(kt == NKT - 1))
            if t == 0:
                # sprinkle pair-1 projections into the exp-bound loop
                if kt % 4 == 3:
                    v_proj_st(qt * 4 + (kt - 3) // 4, 1)
                elif kt % 8 == 5 and qk1_work:
                    kind, pst = qk1_work.pop(0)
                    if kind == "q":
                        qk_proj_st(1, pst, wq_sb, bqp_sb, qT_sb)
                    else:
                        qk_proj_st(1, pst, wk_sb, bkp_sb, kT_sb)
                if (qt, kt) == (3, 15):
                    for h in range(2, 4):
                        nc.vector.memset(v_sb[h][:, :, DH:DH + 1], 1.0)
            if kt < NKT - 1:
                return
            # ---- end of qt: normalize, stats, chunk gather ----
            for i, (h, av) in enumerate(((h0, av0), (h1, av1))):
                row = 2 * (qt % 2) + i
                zt = prb.tile([DH + 1, QT], f32, tag="zt", name=f"zt{t}{qt}{i}")
                nc.vector.tensor_copy(out=zt, in_=av)
                nc.sync.dma_start(out=rb_d[row:row + 1, :],
                                  in_=zt[DH:DH + 1, :])
                rb = prb.tile([64, QT], f32, tag="rb", name=f"rb{t}{qt}{i}")
                nc.sync.dma_start(out=rb,
                                  in_=rb_d[row:row + 1, :].to_broadcast([64, QT]))
                nc.vector.reciprocal_approx_fast(rb, rb)
                zsl = z_sb[h][:, qt * QT:(qt + 1) * QT]
                nc.vector.tensor_mul(zsl, zt[0:DH, :], rb)
                nc.vector.bn_stats(out=bnst[h][:, qt, :], in_=zsl)
            if qt == 3:
                pair_stats(t)
                nc.sync.dma_start(
                    out=agc_in[t][3][0:1, QT:QT + SCC],
                    in_=state[f"msc{t}"][0:1, :].bitcast(bf16))
            nc.sync.dma_start(out=agc_in[t][qt][0:64, 0:QT],
                              in_=z_sb[h0][:, qt * QT:(qt + 1) * QT])
            nc.sync.dma_start(out=agc_in[t][qt][64:128, 0:QT],
                              in_=z_sb[h1][:, qt * QT:(qt + 1) * QT])
            gather(agc_in[t][qt], agc_out[t][qt])
            stage_chunk(t, qt)

        # ---- lead: first q/k tiles, then start the attention pipeline
        qk_proj_pair(0, 0, wq_sb, bqp_sb, qT_sb)
        qk_proj_pair(0, 0, wk_sb, bkp_sb, kT_sb)
        emit_scores_exp(0)
        qk_proj_pair(0, 2, wq_sb, bqp_sb, qT_sb)
        qk_proj_pair(0, 2, wk_sb, bkp_sb, kT_sb)
        for st in range(NKT):
            v_proj_st(st, 0)
        for h in range(2):
            nc.vector.memset(v_sb[h][:, :, DH:DH + 1], 1.0)

        # receiver maps for pair-0 chunks (payload lands mid-pair-1)
        def build_maps(t, sc, pool, ptag):
            scf = sc[:, :, :].bitcast(f32)     # [1, 4, 4]: M0 r0 M1 r1
            mm = pool.tile([128, 4], f32, tag=ptag, name=f"mapm{t}")
            rr = pool.tile([128, 4], f32, tag=ptag, name=f"mapr{t}")
            for o in range(2):
                nc.tensor.matmul(mm[64 * o:64 * (o + 1), :], ones1r,
                                 scf[:, :, 2 * o], start=True, stop=True)
                nc.tensor.matmul(rr[64 * o:64 * (o + 1), :], ones1r,
                                 scf[:, :, 2 * o + 1], start=True, stop=True)
            rmap = pg.tile([128, 4], f32, tag=f"rmap{t}")
            nc.vector.tensor_copy(rmap, rr)
            mvec = pg.tile([128, 4], bf16, tag=f"mvec{t}")
            mtmp = pg.tile([128, 4], f32, tag=f"mtmp{t}")
            nc.vector.tensor_tensor(out=mtmp, in0=bvo_sb[:, t::2], in1=mm,
                                    op=ALU.subtract)
            nc.vector.tensor_mul(mvec, mtmp, rmap)
            wos = pg.tile([128, 4, CW], bf16, tag=f"wos{t}")
            for g in range(4):
                nc.vector.tensor_scalar(out=wos[:, g, :],
                                        in0=wo_sb[:, 2 * g + t, :],
                                        scalar1=rmap[:, g:g + 1], scalar2=None,
                                        op0=ALU.mult)
            return mvec, wos

        pg = ctx.enter_context(tc.tile_pool(name="pg", bufs=1))
        mvec0 = wos0 = None
        for j in range(len(SEQA)):
            attn_iter(j)
            if SEQA[j] == (1, 2, 7):
                mvec0, wos0 = build_maps(0, state["sc160"], ppp, "pp")

        if debug:
            for h in range(HPC):
                nc.gpsimd.dma_start(out=dbgz_d[h], in_=z_sb[h])
                nc.sync.dma_start(out=dbgmr_d[h:h + 1, :], in_=mr_sb[h])
            nc.sync.dma_start(out=dbgnr_d[0], in_=nrmg[:, 0::2, :])
            nc.sync.dma_start(out=dbgnr_d[1], in_=nrmg[:, 1::2, :])

        # ---- tail: pair-1 maps, bias row, out-projection ----
        with tc.tile_pool(name="pystage", bufs=1) as pystage:
            ystage = [pystage.tile([128, S], f32, tag=f"ys{nt}", name=f"ys{nt}")
                      for nt in range(2)]
            # pair-0 accumulation runs while the last gather is in flight
            yp0 = [psc.tile([128, 2 * QT], f32, tag="s", name=f"yp0_{j}")
                   for j in range(2)]
            for g in range(4):
                for st in range(NQT):
                    nc.tensor.matmul(
                        yp0[st // 2][:, (st % 2) * QT:(st % 2 + 1) * QT],
                        wos0[:, g, 0:128],
                        nrmg[:, 2 * g, st * QT:(st + 1) * QT],
                        start=(g == 0), stop=False)
            yp1a = [ppp.tile([128, QT], f32, tag="pp", name=f"yp1a_{st}")
                    for st in range(2)]
            for g in range(4):
                for st in range(2):
                    nc.tensor.matmul(
                        yp1a[st],
                        wos0[:, g, 128:256],
                        nrmg[:, 2 * g, st * QT:(st + 1) * QT],
                        start=(g == 0), stop=False)

            mvec1, wos1 = build_maps(1, state["sc161"], pav, "av")

            # bias row: bo + sum_d (bv-M)*r*wo over all chunks
            cstp = pav.tile([1, CW], f32, tag="av", name="cstp")
            for t, mv in ((0, mvec0), (1, mvec1)):
                for g in range(4):
                    nc.tensor.matmul(cstp, mv[:, g:g + 1], wo_sb[:, 2 * g + t, :],
                                     start=(t == 0 and g == 0),
                                     stop=(t == 1 and g == 3))
            brow = pg.tile([1, CW], bf16, tag="brow")
            nc.vector.tensor_tensor(out=brow, in0=bor_sb, in1=cstp, op=ALU.add)

            # nt0: pair-1 chunks + bias + drain (st3's chunk lands last)
            for g in range(4):
                for st in range(NQT):
                    nc.tensor.matmul(
                        yp0[st // 2][:, (st % 2) * QT:(st % 2 + 1) * QT],
                        wos1[:, g, 0:128],
                        nrmg[:, 2 * g + 1, st * QT:(st + 1) * QT],
                        start=False, stop=False)
            for st in range(NQT):
                src = yp0[st // 2][:, (st % 2) * QT:(st % 2 + 1) * QT]
                nc.tensor.matmul(src, brow[:, 0:128], onesrow,
                                 start=False, stop=True)
                dst = ystage[0][:, st * QT:(st + 1) * QT]
                if st % 2 == 0:
                    nc.scalar.activation(dst, src, AF.Copy)
                else:
                    nc.vector.tensor_copy(out=dst, in_=src)
                nc.sync.dma_start(out=y_d[0, :, st * QT:(st + 1) * QT],
                                  in_=ystage[0][:, st * QT:(st + 1) * QT])

            # nt1: st0-1 in pp psum; st2-3 reuse sc psum after nt0 drains
            for g in range(4):
                for st in range(2):
                    nc.tensor.matmul(yp1a[st], wos1[:, g, 128:256],
                                     nrmg[:, 2 * g + 1, st * QT:(st + 1) * QT],
                                     start=False, stop=False)
            for st in range(2):
                nc.tensor.matmul(yp1a[st], brow[:, 128:256], onesrow,
                                 start=False, stop=True)
            yp1b = psc.tile([128, 2 * QT], f32, tag="s", name="yp1b")
            for g in range(4):
                for st in range(2, NQT):
                    nc.tensor.matmul(
                        yp1b[:, (st - 2) * QT:(st - 1) * QT],
                        wos0[:, g, 128:256],
                        nrmg[:, 2 * g, st * QT:(st + 1) * QT],
                        start=(g == 0), stop=False)
            for g in range(4):
                for st in range(2, NQT):
                    nc.tensor.matmul(
                        yp1b[:, (st - 2) * QT:(st - 1) * QT],
                        wos1[:, g, 128:256],
                        nrmg[:, 2 * g + 1, st * QT:(st + 1) * QT],
                        start=False, stop=False)
            for st in range(2, NQT):
                nc.tensor.matmul(yp1b[:, (st - 2) * QT:(st - 1) * QT],
                                 brow[:, 128:256], onesrow,
                                 start=False, stop=True)
            for st in range(NQT):
                src = (yp1a[st] if st < 2
                       else yp1b[:, (st - 2) * QT:(st - 1) * QT])
                dst = ystage[1][:, st * QT:(st + 1) * QT]
                if st % 2 == 0:
                    nc.scalar.activation(dst, src, AF.Copy)
                else:
                    nc.vector.tensor_copy(out=dst, in_=src)
                nc.sync.dma_start(out=y_d[1, :, st * QT:(st + 1) * QT],
                                  in_=ystage[1][:, st * QT:(st + 1) * QT])

    nc.compile()
    return nc


def _get_nc():
    if "nc" not in _cache:
        _cache["nc"] = _build()
    return _cache["nc"]


def _host_prep(x, Wq, bq, Wk, bk, Wv, bv, Wo, bo, lq1, lk1, lq2, lk2, gn_w, gn_b):
    x = np.asarray(x, np.float32)
    lam = (np.exp((np.asarray(lq1) * np.asarray(lk1)).sum(-1))
           - np.exp((np.asarray(lq2) * np.asarray(lk2)).sum(-1)) + LAMBDA_INIT)
    qscale = (DH ** -0.5) * lam
    Wq_eff = (np.asarray(Wq).reshape(D, H, DH) * qscale[None, :, None]).reshape(D, D)
    bq_eff = (np.asarray(bq).reshape(H, DH) * qscale[:, None]).reshape(D)
    gw = np.asarray(gn_w).reshape(D)
    gb = np.asarray(gn_b).reshape(D)
    Wo_eff = np.asarray(Wo) * gw[:, None]
    bo_eff = np.asarray(bo) + gb @ np.asarray(Wo)
    bk_full = np.asarray(bk)
    bv_full = np.asarray(bv, np.float32)

    # Gathered-row order (chunk (g,t), partition (o,dh) -> head 4g+2t+o) is
    # exactly the original row-major head order, so Wo_eff rows need no
    # permutation.
    xT = np.ascontiguousarray(x.transpose(0, 2, 1))  # [B, D, S]
    bf = ml_dtypes.bfloat16

    def pair_partition_layout(vec256):
        # [256] (head-major: (2t+o)*64+dh) -> [128, 2] with row o*64+dh, col t
        return np.ascontiguousarray(
            vec256.reshape(2, 2, DH).transpose(1, 2, 0).reshape(128, 2)
        ).astype(np.float32)

    # receiver bv map: bvo[o*64+dh, 2g+t] = bv[(4g+2t+o)*64+dh]
    bvo = np.ascontiguousarray(
        bv_full.reshape(4, 2, 2, DH).transpose(2, 3, 0, 1).reshape(128, 8)
    ).astype(np.float32)

    in_maps = []
    for c in range(N_CORES):
        b, hg = c // 4, c % 4
        cs = slice(CW * hg, CW * (hg + 1))
        in_maps.append({
            "xt": np.ascontiguousarray(xT[b]).astype(bf),
            "wq": np.ascontiguousarray(Wq_eff[:, cs]).astype(bf),
            "wk": np.ascontiguousarray(np.asarray(Wk)[:, cs]).astype(bf),
            "wv": np.ascontiguousarray(np.asarray(Wv)[:, cs]).astype(bf),
            "wo": np.ascontiguousarray(Wo_eff[:, cs]).astype(bf),
            "bqp": pair_partition_layout(bq_eff[cs]),
            "bkp": pair_partition_layout(bk_full[cs]),
            "bvh": np.ascontiguousarray(
                bv_full[cs].reshape(HPC, DH).T).astype(np.float32),
            "bvo": bvo,
            "bo": np.ascontiguousarray(bo_eff[cs]).astype(bf),
        })
    return in_maps


def _host_gather(outs):
    # core c=4b+hg produced output columns [256*hg, 256*(hg+1)) as [2,128,S]
    yT = np.empty((B, D, S), np.float32)
    for b in range(B):
        for hg in range(4):
            q = np.asarray(outs[4 * b + hg]["y"]).reshape(CW, S)
            yT[b, CW * hg:CW * (hg + 1), :] = q
    return np.ascontiguousarray(yT.transpose(0, 2, 1))


def kernel(x, Wq, bq, Wk, bk, Wv, bv, Wo, bo, lq1, lk1, lq2, lk2, gn_w, gn_b):
    from concourse.bass_utils import run_bass_kernel_spmd

    in_maps = _host_prep(x, Wq, bq, Wk, bk, Wv, bv, Wo, bo,
                         lq1, lk1, lq2, lk2, gn_w, gn_b)
    nc = _get_nc()
    res = run_bass_kernel_spmd(nc, in_maps, core_ids=list(range(N_CORES)))
    return _host_gather(res.results)


# revision 30
# speedup vs baseline: 1.1723x; 1.0766x over previous
"""Multi-head differential attention on 8 Trainium2 NeuronCores.

Sharding: core c -> batch c//4, head-group c%4 (4 of 16 heads).
Per core: QKV projection for its heads (pair-1 q/k and v sprinkled into
pair-0's exp-bound attention loop), k-major attention (scores via
row-group-packed 64-partition matmul pairs; softmax denominators from a
ones-row appended to V), per-(batch,head,qt-chunk) softmax normalization
pipelined inside the attention loop, GroupNorm statistics via bn_stats
with a DVE Newton rsqrt (no scalar-engine table switch).

Raw (pre-GroupNorm) z is AllGathered in eight per-qt [128,512] chunks
pipelined across both pairs' attention so the slow collective fabric
streams continuously.  Each pair's GN scalars (mean/rstd per head)
travel as bitcast payload: pair 0's ride its last chunk, pair 1's go in
a tiny dedicated gather fired before the last chunk so the Wo-scaling
fold overlaps the final transfer.  The receiver folds (bv-M)*r into a
scaled Wo and a bias row; the out-projection accumulates pair-0 chunks
while the last gather is in flight and finishes per seq-tile.

Host side folds: lambda and softmax scale into Wq/bq; GroupNorm affine
into Wo/bo.  x is pre-transposed per batch and cast to bf16.
"""

import numpy as np
import ml_dtypes

B, S, D, H, DH = 2, 2048, 1024, 16, 64
HPC = 4            # heads per core
CW = HPC * DH      # attention columns per core (256)
EPS = 1e-5
LAMBDA_INIT = 0.8
N_CORES = 8
SCC = 8            # scalar payload columns (4 f32 as 8 bf16)
RSQRT_MAGIC = 1.32118221e+19   # f32 with bits 0x5f3759df

NQT = 4            # query tiles of 512
QT = 512
NKT = 16           # key tiles of 128
NDC = 8            # d-chunks of 128
RG = [[0, 1, 2, 3], [4, 5, 6, 7]]

_cache = {}


def _build(with_collective=True, debug=False):
    from contextlib import ExitStack
    import concourse.bass as bass
    from concourse import bacc
    import concourse.tile as tile
    import concourse.mybir as mybir

    f32 = mybir.dt.float32
    i32 = mybir.dt.int32
    bf16 = mybir.dt.bfloat16
    AF = mybir.ActivationFunctionType
    ALU = mybir.AluOpType

    nc = bacc.Bacc("TRN2", target_bir_lowering=False, debug=False,
                   num_devices=N_CORES)

    xt_d = nc.dram_tensor("xt", [D, S], bf16, kind="ExternalInput")
    wq_d = nc.dram_tensor("wq", [D, CW], bf16, kind="ExternalInput")
    wk_d = nc.dram_tensor("wk", [D, CW], bf16, kind="ExternalInput")
    wv_d = nc.dram_tensor("wv", [D, CW], bf16, kind="ExternalInput")
    # wo: gathered-row layout [(g t o p), quarter-cols]
    wo_d = nc.dram_tensor("wo", [D, CW], bf16, kind="ExternalInput")
    bqp_d = nc.dram_tensor("bqp", [128, 2], f32, kind="ExternalInput")
    bkp_d = nc.dram_tensor("bkp", [128, 2], f32, kind="ExternalInput")
    bvh_d = nc.dram_tensor("bvh", [64, HPC], f32, kind="ExternalInput")
    bvo_d = nc.dram_tensor("bvo", [128, 2 * HPC], f32, kind="ExternalInput")
    bo_d = nc.dram_tensor("bo", [CW], bf16, kind="ExternalInput")
    y_d = nc.dram_tensor("y", [2, 128, S], f32, kind="ExternalOutput")

    # per-(pair,qt) chunk gathers; pair-0 qt3 carries the payload columns
    def chunk_w(t, q):
        return QT + (SCC if (q == 3 and t == 0) else 0)
    agc_in = [[nc.dram_tensor(f"agc_in{t}{q}", [128, chunk_w(t, q)], bf16)
               for q in range(NQT)] for t in range(2)]
    agc_out = [[nc.dram_tensor(f"agc_out{t}{q}", [4, 128, chunk_w(t, q)], bf16)
                for q in range(NQT)] for t in range(2)]
    sc1_in = nc.dram_tensor("sc1_in", [1, SCC], bf16)
    sc1_out = nc.dram_tensor("sc1_out", [4, 1, SCC], bf16)
    rb_d = nc.dram_tensor("rb_bounce", [4, QT], f32)
    if debug:
        dbgz_d = nc.dram_tensor("dbgz", [HPC, DH, S], f32, kind="ExternalOutput")
        dbgmr_d = nc.dram_tensor("dbgmr", [HPC, 2], f32, kind="ExternalOutput")
        dbgnr_d = nc.dram_tensor("dbgnr", [2, 128, 4, S], bf16, kind="ExternalOutput")

    with ExitStack() as ctx:
        tc = ctx.enter_context(tile.TileContext(nc))
        const = ctx.enter_context(tc.tile_pool(name="const", bufs=1))
        big = ctx.enter_context(tc.tile_pool(name="big", bufs=1))
        psc = ctx.enter_context(tc.tile_pool(name="psc", bufs=2, space="PSUM"))
        pav = ctx.enter_context(tc.tile_pool(name="pav", bufs=2, space="PSUM"))
        ppp = ctx.enter_context(tc.tile_pool(name="ppp", bufs=2, space="PSUM"))
        pexp = ctx.enter_context(tc.tile_pool(name="pexp", bufs=4))
        pd = ctx.enter_context(tc.tile_pool(name="pd", bufs=1))
        prb = ctx.enter_context(tc.tile_pool(name="prb", bufs=2))

        # ---- input DMAs: spread across engine queues for bandwidth ----
        wq_sb = const.tile([128, NDC, CW], bf16, tag="wq")
        wk_sb = const.tile([128, NDC, CW], bf16, tag="wk")
        wv_sb = const.tile([128, NDC, CW], bf16, tag="wv")
        wo_sb = const.tile([128, NDC, CW], bf16, tag="wo")
        nc.sync.dma_start(out=wq_sb, in_=wq_d[:, :].rearrange("(c p) n -> p c n", p=128))
        nc.gpsimd.dma_start(out=wk_sb, in_=wk_d[:, :].rearrange("(c p) n -> p c n", p=128))
        nc.scalar.dma_start(out=wv_sb, in_=wv_d[:, :].rearrange("(c p) n -> p c n", p=128))
        nc.scalar.dma_start(out=wo_sb, in_=wo_d[:, :].rearrange("(c p) n -> p c n", p=128))
        pxt = ctx.enter_context(tc.tile_pool(name="pxt", bufs=1))
        xt_sb = [pxt.tile([128, S], bf16, tag=f"xt{c}", name=f"xt{c}")
                 for c in range(NDC)]
        for c in range(NDC):
            eng = nc.sync if c % 2 == 0 else nc.gpsimd
            eng.dma_start(out=xt_sb[c], in_=xt_d[c * 128:(c + 1) * 128, :])

        # small constants on the gpsimd queue
        bqp_sb = const.tile([128, 2], f32, tag="bqp")
        bkp_sb = const.tile([128, 2], f32, tag="bkp")
        bvh_sb = const.tile([64, HPC], f32, tag="bvh")
        bvo_sb = const.tile([128, 2 * HPC], f32, tag="bvo")
        bor_sb = const.tile([1, CW], bf16, tag="bor")
        nc.gpsimd.dma_start(out=bqp_sb, in_=bqp_d[:, :])
        nc.gpsimd.dma_start(out=bkp_sb, in_=bkp_d[:, :])
        nc.gpsimd.dma_start(out=bvh_sb, in_=bvh_d[:, :])
        nc.gpsimd.dma_start(out=bvo_sb, in_=bvo_d[:, :])
        nc.gpsimd.dma_start(out=bor_sb, in_=bo_d[:].rearrange("(a n) -> a n", a=1))

        ones64 = const.tile([64, 1], f32, tag="ones64")
        nc.vector.memset(ones64, 1.0)
        ones1r = const.tile([1, 64], f32, tag="ones1r")
        nc.vector.memset(ones1r, 1.0)
        onesrow = const.tile([1, QT], bf16, tag="onesrow")
        nc.vector.memset(onesrow, 1.0)
        magic_sb = const.tile([1, 2], f32, tag="magic")
        nc.vector.memset(magic_sb, RSQRT_MAGIC)

        # pre-warm the exp activation table while DMAs stream
        warm_sb = const.tile([1, 1], f32, tag="warm")
        nc.vector.memset(warm_sb, 0.0)
        warm2 = const.tile([1, 1], f32, tag="warm2")
        nc.scalar.activation(warm2, warm_sb, AF.Exp)

        qT_sb = big.tile([128, 2, S], bf16, tag="qT")   # pair t; head-parity o rows
        kT_sb = big.tile([128, 2, S], bf16, tag="kT")
        v_sb = [big.tile([128, NKT, DH + 1], bf16, tag=f"v{h}", name=f"v{h}")
                for h in range(HPC)]
        z_sb = [big.tile([DH, S], bf16, tag=f"z{h}", name=f"z{h}")
                for h in range(HPC)]
        nrmg = big.tile([128, NDC, S], bf16, tag="nrmg")   # chunk c=2g+t

        bnst = [pd.tile([64, NQT, 6], f32, tag=f"bn{h}", name=f"bnst{h}")
                for h in range(HPC)]
        stk_all = [pd.tile([64, 3], f32, tag=f"stk{h}", name=f"stk{h}")
                   for h in range(HPC)]
        mr_sb = {}
        state = {}

        def qk_proj_pair(t, st0, w_sb, bp_sb, dst):
            # two query-tile columns, c-outer so work starts on xt chunk 0
            ps = [ppp.tile([128, QT], f32, tag="pp",
                           name=f"qk{t}{st0}{j}{w_sb.tensor.name}")
                  for j in range(2)]
            for c in range(NDC):
                for j in range(2):
                    nc.tensor.matmul(ps[j], w_sb[:, c, t * 128:(t + 1) * 128],
                                     xt_sb[c][:, (st0 + j) * QT:(st0 + j + 1) * QT],
                                     start=(c == 0), stop=(c == NDC - 1))
            for j in range(2):
                nc.vector.tensor_scalar(
                    out=dst[:, t, (st0 + j) * QT:(st0 + j + 1) * QT],
                    in0=ps[j], scalar1=bp_sb[:, t:t + 1],
                    scalar2=None, op0=ALU.add)

        def qk_proj_st(t, st, w_sb, bp_sb, dst):
            ps = ppp.tile([128, QT], f32, tag="pp",
                          name=f"qk{t}{st}{w_sb.tensor.name}")
            for c in range(NDC):
                nc.tensor.matmul(ps, w_sb[:, c, t * 128:(t + 1) * 128],
                                 xt_sb[c][:, st * QT:(st + 1) * QT],
                                 start=(c == 0), stop=(c == NDC - 1))
            nc.vector.tensor_scalar(out=dst[:, t, st * QT:(st + 1) * QT],
                                    in0=ps, scalar1=bp_sb[:, t:t + 1],
                                    scalar2=None, op0=ALU.add)

        def v_proj_st(st, h01):
            # v for heads [2*h01, 2*h01+1] at key-tile st
            cs = slice(h01 * 2 * DH, (h01 + 1) * 2 * DH)
            ps = ppp.tile([128, QT], f32, tag="pp", name=f"v{h01}{st}")
            for c in range(NDC):
                nc.tensor.matmul(ps[:, 0:2 * DH],
                                 xt_sb[c][:, st * 128:(st + 1) * 128],
                                 wv_sb[:, c, cs],
                                 start=(c == 0), stop=(c == NDC - 1))
            for hh in range(2):
                h = 2 * h01 + hh
                nc.vector.tensor_copy(out=v_sb[h][:, st, 0:DH],
                                      in_=ps[:, hh * DH:(hh + 1) * DH])

        def pair_stats(t):
            # bn_aggr + cross-partition combine + Newton rsqrt -> (M', r)
            h0, h1 = 2 * t, 2 * t + 1
            scg = pd.tile([1, 2, 3], f32, tag=f"scg{t}", name=f"scg{t}")
            for i, h in enumerate((h0, h1)):
                mvh = pd.tile([64, 2], f32, tag="mv", bufs=2, name=f"mv{h}")
                nc.vector.bn_aggr(out=mvh, in_=bnst[h])
                stk = stk_all[h]
                nc.vector.tensor_add(stk[:, 0:1], mvh[:, 0:1], bvh_sb[:, h:h + 1])
                nc.vector.tensor_copy(stk[:, 1:2], mvh[:, 1:2])
                nc.vector.tensor_mul(stk[:, 2:3], stk[:, 0:1], stk[:, 0:1])
                stp = ppp.tile([1, 3], f32, tag="pp", name=f"stp{h}")
                nc.tensor.matmul(stp, ones64, stk, start=True, stop=True)
                nc.vector.tensor_scalar(out=scg[:, i, :], in0=stp,
                                        scalar1=1.0 / 64.0, scalar2=None,
                                        op0=ALU.mult)
            # var_tot = E[var] + E[(m+bv)^2] - M'^2 ; r = rsqrt(var_tot + eps)
            m2 = pd.tile([1, 2], f32, tag=f"m2{t}", name=f"m2{t}")
            nc.vector.tensor_mul(m2, scg[:, :, 0], scg[:, :, 0])
            vr = pd.tile([1, 2], f32, tag=f"vr{t}", name=f"vr{t}")
            nc.vector.tensor_add(vr, scg[:, :, 1], scg[:, :, 2])
            nc.vector.tensor_tensor(out=vr, in0=vr, in1=m2, op=ALU.subtract)
            nc.vector.tensor_scalar(out=vr, in0=vr, scalar1=EPS, scalar2=None,
                                    op0=ALU.add)
            yr = pd.tile([1, 2], f32, tag=f"yr{t}", name=f"yr{t}")
            ish = pd.tile([1, 2], i32, tag=f"ish{t}", name=f"ish{t}")
            nc.vector.tensor_scalar(out=ish, in0=vr[:, :].bitcast(i32),
                                    scalar1=1, scalar2=None,
                                    op0=ALU.logical_shift_right)
            nc.vector.tensor_tensor(
                out=yr[:, :].bitcast(i32), in0=magic_sb[:, :].bitcast(i32),
                in1=ish, op=ALU.subtract)
            tt = pd.tile([1, 2], f32, tag=f"tt{t}", name=f"tt{t}")
            for _ in range(2):
                nc.vector.tensor_mul(tt, yr, yr)
                nc.vector.tensor_mul(tt, tt, vr)
                nc.vector.tensor_scalar(out=tt, in0=tt, scalar1=-0.5,
                                        scalar2=1.5, op0=ALU.mult, op1=ALU.add)
                nc.vector.tensor_mul(yr, yr, tt)
            for i, h in enumerate((h0, h1)):
                mr = pd.tile([1, 2], f32, tag="mr", bufs=4, name=f"mr{h}")
                nc.vector.tensor_copy(mr[:, 0:1], scg[:, i, 0:1])
                nc.vector.tensor_copy(mr[:, 1:2], yr[:, i:i + 1])
                mr_sb[h] = mr
            # payload [M_h0, r_h0, M_h1, r_h1]
            msc = pd.tile([1, 4], f32, tag=f"msc{t}", name=f"msc{t}")
            for i, h in enumerate((h0, h1)):
                nc.vector.tensor_copy(msc[:, 2 * i:2 * i + 1], mr_sb[h][:, 0:1])
                nc.vector.tensor_copy(msc[:, 2 * i + 1:2 * i + 2],
                                      mr_sb[h][:, 1:2])
            state[f"msc{t}"] = msc

        def gather(in_t, out_t):
            if with_collective:
                nc.gpsimd.collective_compute(
                    "AllGather", ALU.bypass, replica_groups=RG,
                    ins=[in_t[:].opt()], outs=[out_t[:].opt()])
            else:
                for g in range(4):
                    nc.sync.dma_start(out=out_t[g], in_=in_t[:, :])

        def stage_chunk(t, q):
            # gathered chunk -> nrmg[:, 2g+t, q-slice]; then payload extract
            nc.gpsimd.dma_start(
                out=nrmg[:, t::2, q * QT:(q + 1) * QT],
                in_=agc_out[t][q][:, :, 0:QT].rearrange("g p q -> p g q"))
            if q == 3 and t == 0:
                sc = pd.tile([1, 4, SCC], bf16, tag="sc160", name="sc160")
                nc.gpsimd.dma_start(
                    out=sc,
                    in_=agc_out[0][3][:, 0:1, QT:QT + SCC].rearrange("g p c -> p g c"))
                state["sc160"] = sc

        qk1_work = []
        for st in range(NQT):
            qk1_work.append(("q", st))
            qk1_work.append(("k", st))

        SEQA = [(t, qt, kt) for t in (0, 1) for qt in range(NQT)
                for kt in range(NKT)]
        e_tiles = {}
        av_cur = {}

        def emit_scores_exp(j):
            t, qt, kt = SEQA[j]
            sps = psc.tile([128, 2 * QT], f32, tag="s", name=f"s{t}{qt}{kt}")
            for o in range(2):
                nc.tensor.matmul(
                    sps[:, o * QT:(o + 1) * QT],
                    kT_sb[64 * o:64 * (o + 1), t, kt * 128:(kt + 1) * 128],
                    qT_sb[64 * o:64 * (o + 1), t, qt * QT:(qt + 1) * QT],
                    start=True, stop=True)
            e_sb = pexp.tile([128, 2 * QT], bf16, tag="e", name=f"e{t}{qt}{kt}")
            nc.scalar.activation(e_sb, sps, AF.Exp)
            e_tiles[j] = e_sb

        def attn_iter(j):
            t, qt, kt = SEQA[j]
            h0, h1 = 2 * t, 2 * t + 1
            if j + 1 < len(SEQA):
                emit_scores_exp(j + 1)
            if kt == 0:
                av_cur[0] = pav.tile([DH + 1, QT], f32, tag="av", name=f"av{t}{qt}a")
                av_cur[1] = pav.tile([DH + 1, QT], f32, tag="av", name=f"av{t}{qt}b")
            av0, av1 = av_cur[0], av_cur[1]
            e_sb = e_tiles.pop(j)
            nc.tensor.matmul(av0, v_sb[h0][:, kt, :], e_sb[:, 0:QT],
                             start=(kt == 0), stop=(kt == NKT - 1))
            nc.tensor.matmul(av1, v_sb[h1][:, kt, :], e_sb[:, QT:2 * QT],
                             start=(kt == 0), stop=(kt == NKT - 1))
            if t == 0:
                # sprinkle pair-1 projections into the exp-bound loop
                if kt % 4 == 3:
                    v_proj_st(qt * 4 + (kt - 3) // 4, 1)
                elif kt % 8 == 5 and qk1_work:
                    kind, pst = qk1_work.pop(0)
                    if kind == "q":
                        qk_proj_st(1, pst, wq_sb, bqp_sb, qT_sb)
                    else:
                        qk_proj_st(1, pst, wk_sb, bkp_sb, kT_sb)
                if (qt, kt) == (3, 15):
                    for h in range(2, 4):
                        nc.vector.memset(v_sb[h][:, :, DH:DH + 1], 1.0)
            if kt < NKT - 1:
                return
            # ---- end of qt: normalize, stats, chunk gather ----
            for i, (h, av) in enumerate(((h0, av0), (h1, av1))):
                row = 2 * (qt % 2) + i
                zt = prb.tile([DH + 1, QT], f32, tag="zt", name=f"zt{t}{qt}{i}")
                nc.vector.tensor_copy(out=zt, in_=av)
                nc.sync.dma_start(out=rb_d[row:row + 1, :],
                                  in_=zt[DH:DH + 1, :])
                rb = prb.tile([64, QT], f32, tag="rb", name=f"rb{t}{qt}{i}")
                nc.sync.dma_start(out=rb,
                                  in_=rb_d[row:row + 1, :].to_broadcast([64, QT]))
                nc.vector.reciprocal_approx_fast(rb, rb)
                zsl = z_sb[h][:, qt * QT:(qt + 1) * QT]
                nc.vector.tensor_mul(zsl, zt[0:DH, :], rb)
                nc.vector.bn_stats(out=bnst[h][:, qt, :], in_=zsl)
            if qt == 3 and t == 0:
                pair_stats(0)
                nc.sync.dma_start(
                    out=agc_in[0][3][0:1, QT:QT + SCC],
                    in_=state["msc0"][0:1, :].bitcast(bf16))
            nc.sync.dma_start(out=agc_in[t][qt][0:64, 0:QT],
                              in_=z_sb[h0][:, qt * QT:(qt + 1) * QT])
            nc.sync.dma_start(out=agc_in[t][qt][64:128, 0:QT],
                              in_=z_sb[h1][:, qt * QT:(qt + 1) * QT])
            gather(agc_in[t][qt], agc_out[t][qt])
            stage_chunk(t, qt)
            if qt == 3 and t == 1:
                # pair-1 scalars in a tiny follow-up gather; its DMA is not
                # queued ahead of the z chunk
                pair_stats(1)
                nc.sync.dma_start(
                    out=sc1_in[0:1, :],
                    in_=state["msc1"][0:1, :].bitcast(bf16))
                gather(sc1_in, sc1_out)
                scb = pd.tile([1, 4, SCC], bf16, tag="sc161", name="sc161")
                nc.gpsimd.dma_start(
                    out=scb,
                    in_=sc1_out[:, :, :].rearrange("g p c -> p g c"))
                state["sc161"] = scb

        # ---- lead: q/k st0-1 c-interleaved in psc halves, early pipeline
        qk01 = psc.tile([128, 2 * QT], f32, tag="s", name="qk01q")
        kk01 = psc.tile([128, 2 * QT], f32, tag="s", name="qk01k")
        for c in range(NDC):
            for j in range(2):
                nc.tensor.matmul(qk01[:, j * QT:(j + 1) * QT],
                                 wq_sb[:, c, 0:128],
                                 xt_sb[c][:, j * QT:(j + 1) * QT],
                                 start=(c == 0), stop=(c == NDC - 1))
            for j in range(2):
                nc.tensor.matmul(kk01[:, j * QT:(j + 1) * QT],
                                 wk_sb[:, c, 0:128],
                                 xt_sb[c][:, j * QT:(j + 1) * QT],
                                 start=(c == 0), stop=(c == NDC - 1))
        for j in range(2):
            nc.vector.tensor_scalar(out=qT_sb[:, 0, j * QT:(j + 1) * QT],
                                    in0=qk01[:, j * QT:(j + 1) * QT],
                                    scalar1=bqp_sb[:, 0:1],
                                    scalar2=None, op0=ALU.add)
            nc.vector.tensor_scalar(out=kT_sb[:, 0, j * QT:(j + 1) * QT],
                                    in0=kk01[:, j * QT:(j + 1) * QT],
                                    scalar1=bkp_sb[:, 0:1],
                                    scalar2=None, op0=ALU.add)
        emit_scores_exp(0)
        qk_proj_pair(0, 2, wq_sb, bqp_sb, qT_sb)
        qk_proj_pair(0, 2, wk_sb, bkp_sb, kT_sb)
        for st in range(NKT):
            v_proj_st(st, 0)
        for h in range(2):
            nc.vector.memset(v_sb[h][:, :, DH:DH + 1], 1.0)

        # receiver maps for pair-0 chunks (payload lands mid-pair-1)
        def build_maps(t, sc, pool, ptag):
            scf = sc[:, :, :].bitcast(f32)     # [1, 4, 4]: M0 r0 M1 r1
            mm = pool.tile([128, 4], f32, tag=ptag, name=f"mapm{t}")
            rr = pool.tile([128, 4], f32, tag=ptag, name=f"mapr{t}")
            for o in range(2):
                nc.tensor.matmul(mm[64 * o:64 * (o + 1), :], ones1r,
                                 scf[:, :, 2 * o], start=True, stop=True)
                nc.tensor.matmul(rr[64 * o:64 * (o + 1), :], ones1r,
                                 scf[:, :, 2 * o + 1], start=True, stop=True)
            rmap = pg.tile([128, 4], f32, tag=f"rmap{t}")
            nc.vector.tensor_copy(rmap, rr)
            mvec = pg.tile([128, 4], bf16, tag=f"mvec{t}")
            mtmp = pg.tile([128, 4], f32, tag=f"mtmp{t}")
            nc.vector.tensor_tensor(out=mtmp, in0=bvo_sb[:, t::2], in1=mm,
                                    op=ALU.subtract)
            nc.vector.tensor_mul(mvec, mtmp, rmap)
            wos = pg.tile([128, 4, CW], bf16, tag=f"wos{t}")
            for g in range(4):
                nc.vector.tensor_scalar(out=wos[:, g, :],
                                        in0=wo_sb[:, 2 * g + t, :],
                                        scalar1=rmap[:, g:g + 1], scalar2=None,
                                        op0=ALU.mult)
            return mvec, wos

        pg = ctx.enter_context(tc.tile_pool(name="pg", bufs=1))
        mvec0 = wos0 = None
        for j in range(len(SEQA)):
            attn_iter(j)
            if SEQA[j] == (1, 2, 7):
                mvec0, wos0 = build_maps(0, state["sc160"], ppp, "pp")

        if debug:
            for h in range(HPC):
                nc.gpsimd.dma_start(out=dbgz_d[h], in_=z_sb[h])
                nc.sync.dma_start(out=dbgmr_d[h:h + 1, :], in_=mr_sb[h])
            nc.sync.dma_start(out=dbgnr_d[0], in_=nrmg[:, 0::2, :])
            nc.sync.dma_start(out=dbgnr_d[1], in_=nrmg[:, 1::2, :])

        # ---- tail: pair-1 maps, bias row, out-projection ----
        with tc.tile_pool(name="pystage", bufs=1) as pystage:
            ystage = [pystage.tile([128, S], f32, tag=f"ys{nt}", name=f"ys{nt}")
                      for nt in range(2)]
            # pair-0 accumulation runs while the last gather is in flight
            yp0 = [psc.tile([128, 2 * QT], f32, tag="s", name=f"yp0_{j}")
                   for j in range(2)]
            for g in range(4):
                for st in range(NQT):
                    nc.tensor.matmul(
                        yp0[st // 2][:, (st % 2) * QT:(st % 2 + 1) * QT],
                        wos0[:, g, 0:128],
                        nrmg[:, 2 * g, st * QT:(st + 1) * QT],
                        start=(g == 0), stop=False)
            yp1a = [ppp.tile([128, QT], f32, tag="pp", name=f"yp1a_{st}")
                    for st in range(2)]
            for g in range(4):
                for st in range(2):
                    nc.tensor.matmul(
                        yp1a[st],
                        wos0[:, g, 128:256],
                        nrmg[:, 2 * g, st * QT:(st + 1) * QT],
                        start=(g == 0), stop=False)

            mvec1, wos1 = build_maps(1, state["sc161"], pav, "av")

            # bias row: bo + sum_d (bv-M)*r*wo over all chunks
            cstp = pav.tile([1, CW], f32, tag="av", name="cstp")
            for t, mv in ((0, mvec0), (1, mvec1)):
                for g in range(4):
                    nc.tensor.matmul(cstp, mv[:, g:g + 1], wo_sb[:, 2 * g + t, :],
                                     start=(t == 0 and g == 0),
                                     stop=(t == 1 and g == 3))
            brow = pg.tile([1, CW], bf16, tag="brow")
            nc.vector.tensor_tensor(out=brow, in0=bor_sb, in1=cstp, op=ALU.add)

            # nt0: pair-1 chunks + bias + drain (st3's chunk lands last)
            for g in range(4):
                for st in range(NQT):
                    nc.tensor.matmul(
                        yp0[st // 2][:, (st % 2) * QT:(st % 2 + 1) * QT],
                        wos1[:, g, 0:128],
                        nrmg[:, 2 * g + 1, st * QT:(st + 1) * QT],
                        start=False, stop=False)
            for st in range(NQT):
                src = yp0[st // 2][:, (st % 2) * QT:(st % 2 + 1) * QT]
                nc.tensor.matmul(src, brow[:, 0:128], onesrow,
                                 start=False, stop=True)
                dst = ystage[0][:, st * QT:(st + 1) * QT]
                if st % 2 == 0:
                    nc.scalar.activation(dst, src, AF.Copy)
                else:
                    nc.vector.tensor_copy(out=dst, in_=src)
                nc.sync.dma_start(out=y_d[0, :, st * QT:(st + 1) * QT],
                                  in_=ystage[0][:, st * QT:(st + 1) * QT])

            # nt1: st0-1 in pp psum; st2-3 reuse sc psum after nt0 drains
            for g in range(4):
                for st in range(2):
                    nc.tensor.matmul(yp1a[st], wos1[:, g, 128:256],
                                     nrmg[:, 2 * g + 1, st * QT:(st + 1) * QT],
                                     start=False, stop=False)
            for st in range(2):
                nc.tensor.matmul(yp1a[st], brow[:, 128:256], onesrow,
                                 start=False, stop=True)
            yp1b = psc.tile([128, 2 * QT], f32, tag="s", name="yp1b")
            for g in range(4):
                for st in range(2, NQT):
                    nc.tensor.matmul(
                        yp1b[:, (st - 2) * QT:(st - 1) * QT],
                        wos0[:, g, 128:256],
                        nrmg[:, 2 * g, st * QT:(st + 1) * QT],
                        start=(g == 0), stop=False)
            for g in range(4):
                for st in range(2, NQT):
                    nc.tensor.matmul(
                        yp1b[:, (st - 2) * QT:(st - 1) * QT],
                        wos1[:, g, 128:256],
                        nrmg[:, 2 * g + 1, st * QT:(st + 1) * QT],
                        start=False, stop=False)
            for st in range(2, NQT):
                nc.tensor.matmul(yp1b[:, (st - 2) * QT:(st - 1) * QT],
                                 brow[:, 128:256], onesrow,
                                 start=False, stop=True)
            for st in range(NQT):
                src = (yp1a[st] if st < 2
                       else yp1b[:, (st - 2) * QT:(st - 1) * QT])
                dst = ystage[1][:, st * QT:(st + 1) * QT]
                if st % 2 == 0:
                    nc.scalar.activation(dst, src, AF.Copy)
                else:
                    nc.vector.tensor_copy(out=dst, in_=src)
                nc.sync.dma_start(out=y_d[1, :, st * QT:(st + 1) * QT],
                                  in_=ystage[1][:, st * QT:(st + 1) * QT])

    nc.compile()
    return nc


def _get_nc():
    if "nc" not in _cache:
        _cache["nc"] = _build()
    return _cache["nc"]


def _host_prep(x, Wq, bq, Wk, bk, Wv, bv, Wo, bo, lq1, lk1, lq2, lk2, gn_w, gn_b):
    x = np.asarray(x, np.float32)
    lam = (np.exp((np.asarray(lq1) * np.asarray(lk1)).sum(-1))
           - np.exp((np.asarray(lq2) * np.asarray(lk2)).sum(-1)) + LAMBDA_INIT)
    qscale = (DH ** -0.5) * lam
    Wq_eff = (np.asarray(Wq).reshape(D, H, DH) * qscale[None, :, None]).reshape(D, D)
    bq_eff = (np.asarray(bq).reshape(H, DH) * qscale[:, None]).reshape(D)
    gw = np.asarray(gn_w).reshape(D)
    gb = np.asarray(gn_b).reshape(D)
    Wo_eff = np.asarray(Wo) * gw[:, None]
    bo_eff = np.asarray(bo) + gb @ np.asarray(Wo)
    bk_full = np.asarray(bk)
    bv_full = np.asarray(bv, np.float32)

    # Gathered-row order (chunk (g,t), partition (o,dh) -> head 4g+2t+o) is
    # exactly the original row-major head order, so Wo_eff rows need no
    # permutation.
    xT = np.ascontiguousarray(x.transpose(0, 2, 1))  # [B, D, S]
    bf = ml_dtypes.bfloat16

    def pair_partition_layout(vec256):
        # [256] (head-major: (2t+o)*64+dh) -> [128, 2] with row o*64+dh, col t
        return np.ascontiguousarray(
            vec256.reshape(2, 2, DH).transpose(1, 2, 0).reshape(128, 2)
        ).astype(np.float32)

    # receiver bv map: bvo[o*64+dh, 2g+t] = bv[(4g+2t+o)*64+dh]
    bvo = np.ascontiguousarray(
        bv_full.reshape(4, 2, 2, DH).transpose(2, 3, 0, 1).reshape(128, 8)
    ).astype(np.float32)

    in_maps = []
    for c in range(N_CORES):
        b, hg = c // 4, c % 4
        cs = slice(CW * hg, CW * (hg + 1))
        in_maps.append({
            "xt": np.ascontiguousarray(xT[b]).astype(bf),
            "wq": np.ascontiguousarray(Wq_eff[:, cs]).astype(bf),
            "wk": np.ascontiguousarray(np.asarray(Wk)[:, cs]).astype(bf),
            "wv": np.ascontiguousarray(np.asarray(Wv)[:, cs]).astype(bf),
            "wo": np.ascontiguousarray(Wo_eff[:, cs]).astype(bf),
            "bqp": pair_partition_layout(bq_eff[cs]),
            "bkp": pair_partition_layout(bk_full[cs]),
            "bvh": np.ascontiguousarray(
                bv_full[cs].reshape(HPC, DH).T).astype(np.float32),
            "bvo": bvo,
            "bo": np.ascontiguousarray(bo_eff[cs]).astype(bf),
        })
    return in_maps


def _host_gather(outs):
    # core c=4b+hg produced output columns [256*hg, 256*(hg+1)) as [2,128,S]
    yT = np.empty((B, D, S), np.float32)
    for b in range(B):
        for hg in range(4):
            q = np.asarray(outs[4 * b + hg]["y"]).reshape(CW, S)
            yT[b, CW * hg:CW * (hg + 1), :] = q
    return np.ascontiguousarray(yT.transpose(0, 2, 1))


def kernel(x, Wq, bq, Wk, bk, Wv, bv, Wo, bo, lq1, lk1, lq2, lk2, gn_w, gn_b):
    from concourse.bass_utils import run_bass_kernel_spmd

    in_maps = _host_prep(x, Wq, bq, Wk, bk, Wv, bv, Wo, bo,
                         lq1, lk1, lq2, lk2, gn_w, gn_b)
    nc = _get_nc()
    res = run_bass_kernel_spmd(nc, in_maps, core_ids=list(range(N_CORES)))
    return _host_gather(res.results)
